# revision 19
# baseline (speedup 1.0000x reference)
"""Trainium2 Bass kernel for nn_ScoreGraphReconstructor (3-layer GATv2 + edge MLP).

Sharding: nodes are permuted into 8*WPC windows of 128 slots, balanced by
in-degree. Each core owns WPC windows (contiguous slot range) and all edges
whose *target* lands in its windows, so the segment softmax/scatter is fully
core-local. Per layer: node matmuls on the local shard -> AllGather of the
source-side transform xl (bf16) -> edge phase (one dma_gather of xl[src],
xr[dst] reconstructed via one-hot transpose matmuls from SBUF, attention on
DVE/ACT, one-hot scatter matmul into PSUM). The edge classifier rides the same
window layout: per-node a/b = h3 @ Wc1 halves, AllGather a, transposed gather
of a[src] with the same indices, b[dst] via the same one-hots; z2/z3 are
feature-major matmuls. All matmuls bf16 (fp32 is 4x slower on the PE).
"""

import sys

for _p in ("/opt/trn_rl_repo",):
    if _p not in sys.path:
        sys.path.insert(0, _p)

import numpy as np
from ml_dtypes import bfloat16 as np_bf16

import concourse.bass as bass
import concourse.bacc as bacc
import concourse.mybir as mybir
import concourse.tile as tile
from concourse.bass_utils import run_bass_kernel_spmd

F32 = mybir.dt.float32
F32R = mybir.dt.float32r
BF16 = mybir.dt.bfloat16
I16 = mybir.dt.int16

NCORES = 8
H, C = 4, 64
HID = H * C  # 256
NCLS = 5
ACT = mybir.ActivationFunctionType
ALU = mybir.AluOpType


class Cfg:
    def __init__(self, n_nodes, n_edges, wpc, T, in_dim=256):
        self.n = n_nodes
        self.e = n_edges
        self.wpc = wpc                    # windows per core
        self.T = T                        # edge tiles (of 128) per window
        self.L = wpc * 128                # local slots per core
        self.S = NCORES * self.L          # total slots
        self.nwin = NCORES * self.wpc
        self.ew = T * 128                 # edge slots per window
        self.in_dim = in_dim


# ---------------------------------------------------------------- host prep


def _balance_windows(deg, nwin):
    """Assign node n (with weight deg[n]) to one of nwin windows, each holding
    exactly 128 nodes (rest dummy), minimizing max window load. Greedy LPT."""
    import heapq

    n = len(deg)
    order = np.argsort(-deg, kind="stable")
    heap = [(0, w) for w in range(nwin)]
    heapq.heapify(heap)
    count = np.zeros(nwin, np.int64)
    slot_of = np.empty(n, np.int64)
    pos = np.zeros(nwin, np.int64)
    for node in order:
        while True:
            load, w = heapq.heappop(heap)
            if count[w] < 128:
                break
        slot_of[node] = w * 128 + pos[w]
        pos[w] += 1
        count[w] += 1
        if count[w] < 128:
            heapq.heappush(heap, (load + int(deg[node]), w))
    return slot_of


def _wrap_idx16(idx, rows=128):
    """dma_gather index layout: index i lives at [i % 16, i // 16] of a
    (rows, len/16) int16 SBUF tile; rows 16..127 padded with copies."""
    n = len(idx)
    assert n % 16 == 0
    blk = np.asarray(idx, np.int16).reshape(n // 16, 16).T
    return np.tile(blk, (rows // 16, 1))


def prepare_host(inputs, cfg):
    """Build per-core input maps + metadata. inputs: dict from setup_inputs."""
    cn = {k: np.asarray(v) for k, v in inputs.items()}
    x = cn["x"].astype(np.float32)
    ei = cn["edge_index"].astype(np.int64)
    row, col = ei[0], ei[1]
    n, e = cfg.n, cfg.e
    L, S, wpc, T = cfg.L, cfg.S, cfg.wpc, cfg.T
    ew = cfg.ew

    deg = np.bincount(col, minlength=n) + 1  # +1 self loop
    slot_of = _balance_windows(deg, cfg.nwin)

    # --- edge lists (conv graph: original edges + self loops on ALL slots)
    all_slots = np.arange(S, dtype=np.int64)
    src_sl = np.concatenate([slot_of[row], all_slots])
    dst_sl = np.concatenate([slot_of[col], all_slots])
    orig_id = np.concatenate(
        [np.arange(e, dtype=np.int64), np.full(S, -1, np.int64)]
    )
    win = dst_sl // 128
    ordr = np.argsort(win, kind="stable")
    src_sl, dst_sl, win, orig_id = (
        src_sl[ordr], dst_sl[ordr], win[ordr], orig_id[ordr]
    )
    starts = np.searchsorted(win, np.arange(cfg.nwin))
    ends = np.searchsorted(win, np.arange(cfg.nwin), side="right")
    maxcnt = int((ends - starts).max())
    assert maxcnt <= ew, f"window overflow: {maxcnt} > {ew}; raise T"

    # per-core edge tensors + output maps
    per_core = []
    out_maps = []
    for c in range(NCORES):
        esrc = np.zeros((128, wpc, ew // 16), np.int16)
        doff = np.full((128, wpc, T), 128, np.float32)
        poss, origs = [], []
        for wi in range(wpc):
            w = c * wpc + wi
            s0, s1 = starts[w], ends[w]
            cnt = s1 - s0
            srcw = np.zeros(ew, np.int64)
            dofw = np.full(ew, 128, np.int64)
            srcw[:cnt] = src_sl[s0:s1]
            dofw[:cnt] = dst_sl[s0:s1] - w * 128
            esrc[:, wi, :] = _wrap_idx16(srcw)
            # edge j -> partition j%128, tile j//128 (both gather + doff)
            doff[:, wi, :] = dofw.reshape(T, 128).T
            oid = orig_id[s0:s1]
            real = oid >= 0
            poss.append(wi * ew + np.nonzero(real)[0])
            origs.append(oid[real])
        # one-hot transposed: stT[nslot, wi, t, p] = (doff[p, wi, t] == nslot)
        stT = (
            np.arange(128, dtype=np.int32)[:, None, None, None]
            == doff[None].transpose(0, 2, 3, 1)
        ).astype(np_bf16)
        per_core.append(
            {
                "esrc": esrc,
                "doff": doff.astype(np_bf16),
                "stT": stT,
            }
        )
        out_maps.append(
            (np.concatenate(poss), np.concatenate(origs))
        )

    # --- node features, permuted + transposed
    xp = np.zeros((S, cfg.in_dim), np.float32)
    xp[slot_of] = x
    nin = cfg.in_dim // 128

    # --- weights with ELU(-1) folds
    wmaps = {}
    for l in (1, 2, 3):
        kin = cfg.in_dim if l == 1 else HID
        Wl = cn[f"W{l}l"].astype(np.float32)
        Wr = cn[f"W{l}r"].astype(np.float32)
        bl = cn[f"b{l}l"].astype(np.float32).copy()
        br = cn[f"b{l}r"].astype(np.float32).copy()
        att = cn[f"att{l}"].astype(np.float32)
        wmaps[f"Wl{l}"] = Wl.reshape(kin // 128, 128, HID).astype(np_bf16)
        wmaps[f"Wr{l}"] = Wr.reshape(kin // 128, 128, HID).astype(np_bf16)
        wmaps[f"bl{l}"] = bl.reshape(1, HID)
        wmaps[f"br{l}"] = br.reshape(1, HID)
        wmaps[f"attb{l}"] = np.tile(att.reshape(1, HID), (128, 1)).astype(np_bf16)
        wmaps[f"biasb{l}"] = np.tile(
            cn[f"bias{l}"].astype(np.float32).reshape(1, HID), (128, 1)
        )
    Wc1 = cn["Wc1"].astype(np.float32)
    Wc2 = cn["Wc2"].astype(np.float32)
    Wc3 = cn["Wc3"].astype(np.float32)
    wmaps["Wc1t"] = Wc1[:HID].reshape(2, 128, HID).astype(np_bf16)
    wmaps["Wc1b"] = Wc1[HID:].reshape(2, 128, HID).astype(np_bf16)
    wmaps["bc1"] = cn["bc1"].astype(np.float32).reshape(1, HID)
    wmaps["Wc2"] = Wc2.reshape(2, 128, HID // 2).astype(np_bf16)
    wmaps["Wc3"] = Wc3.astype(np_bf16)  # [128, 5]
    bc2f = cn["bc2"].astype(np.float32)
    bc3f = cn["bc3"].astype(np.float32)
    wmaps["bc2c"] = bc2f.reshape(HID // 2, 1)
    wmaps["nbc2c"] = (-bc2f).reshape(HID // 2, 1)
    wmaps["bc3c"] = bc3f.reshape(NCLS, 1)
    wmaps["ones"] = np.ones((1, 512), np.float32)
    wmaps["identf"] = np.eye(128, dtype=np.float32)
    wmaps["iotar"] = np.tile(
        np.arange(128, dtype=np.float32), (128, 1)
    ).astype(np_bf16)

    in_maps = []
    for c in range(NCORES):
        m = dict(wmaps)
        m.update(per_core[c])
        xc = xp[c * L : (c + 1) * L].T.copy()  # (in_dim, L)
        m["xfm"] = xc.reshape(nin, 128, L).astype(np_bf16)
        in_maps.append(m)

    flags = (
        tuple(bool(np.any(cn[f"b{l}l"]) or np.any(cn[f"b{l}r"])) for l in (1, 2, 3)),
        tuple(bool(np.any(cn[f"bias{l}"])) for l in (1, 2, 3)),
        bool(np.any(cn["bc1"])),
    )
    meta = {"slot_of": slot_of, "cfg": cfg, "out_maps": out_maps,
            "flags": flags}
    return in_maps, meta


# ---------------------------------------------------------------- device build


def build_nc(cfg, flags=((True,) * 3, (True,) * 3, True), debug=False):
    nc = bacc.Bacc("TRN2", target_bir_lowering=False, debug=debug, num_devices=NCORES)
    has_nbias, has_gbias, has_bc1 = flags
    L, S, wpc, T, ew = cfg.L, cfg.S, cfg.wpc, cfg.T, cfg.ew
    nin = cfg.in_dim // 128
    EC = 384                  # classifier edge-chunk (z1/z2/z3 psum width)
    NEC = ew // EC
    assert ew % EC == 0 and EC % 128 == 0

    P = {}

    def pin(name, shape, dtype=F32):
        P[name] = nc.declare_dram_parameter(name, list(shape), dtype, isOutput=False)

    pin("xfm", (nin, 128, L), BF16)
    pin("esrc", (128, wpc, ew // 16), I16)
    pin("doff", (128, wpc, T), BF16)
    pin("stT", (128, wpc, T, 128), BF16)
    for l in (1, 2, 3):
        nk = nin if l == 1 else 2
        pin(f"Wl{l}", (nk, 128, HID), BF16)
        pin(f"Wr{l}", (nk, 128, HID), BF16)
        pin(f"bl{l}", (1, HID))
        pin(f"br{l}", (1, HID))
        pin(f"attb{l}", (128, HID), BF16)
        pin(f"biasb{l}", (128, HID))
    pin("Wc1t", (2, 128, HID), BF16)
    pin("Wc1b", (2, 128, HID), BF16)
    pin("bc1", (1, HID))
    pin("Wc2", (2, 128, HID // 2), BF16)
    pin("Wc3", (128, NCLS), BF16)
    pin("bc2c", (HID // 2, 1))
    pin("nbc2c", (HID // 2, 1))
    pin("bc3c", (NCLS, 1))
    pin("ones", (1, 512))
    pin("identf", (128, 128))
    pin("iotar", (128, 128), BF16)
    out_t = nc.declare_dram_parameter(
        "out_t", [NCLS, wpc * ew], F32, isOutput=True
    )

    rg = [list(range(NCORES))]

    with tile.TileContext(nc) as tc:
        with (
            tc.tile_pool(name="const", bufs=1) as cp,
            tc.tile_pool(name="dram", bufs=1, space="DRAM") as dp,
            tc.tile_pool(name="work", bufs=2) as wp,
            tc.tile_pool(name="epbig", bufs=3) as ep,
            tc.tile_pool(name="ep1", bufs=1) as ep1,
            tc.tile_pool(name="psn", bufs=2, space="PSUM") as psn,
            tc.tile_pool(name="pso", bufs=2, space="PSUM") as pso,
            tc.tile_pool(name="psc", bufs=2, space="PSUM") as psc,
        ):
            # ---------- constants into SBUF
            def load_const(name, dtype=F32, chunked=False):
                src = P[name]
                shp = list(src.shape)
                if chunked:
                    # [nk, 128, D] DRAM -> [128, nk, D] SBUF (weight chunks)
                    t = cp.tile([shp[1], shp[0], shp[2]], dtype, tag=name,
                                name=name + "_sb")
                    for c in range(shp[0]):
                        nc.sync.dma_start(t[:, c, :], src[c])
                else:
                    t = cp.tile(shp, dtype, tag=name, name=name + "_sb")
                    nc.sync.dma_start(t[:], src[:])
                return t

            xfm = [
                cp.tile([128, L], BF16, tag=f"xfm{c}", name=f"xfm{c}")
                for c in range(nin)
            ]
            for c in range(nin):
                nc.sync.dma_start(xfm[c][:], P["xfm"][c])
            esrc_sb = load_const("esrc", dtype=I16)
            doff_sb = load_const("doff", dtype=BF16)
            stT_sb = load_const("stT", dtype=BF16)
            consts = {}
            for l in (1, 2, 3):
                for nm in (f"Wl{l}", f"Wr{l}"):
                    consts[nm] = load_const(nm, dtype=BF16, chunked=True)
                consts[f"attb{l}"] = load_const(f"attb{l}", dtype=BF16)
                if has_nbias[l - 1]:
                    for nm in (f"bl{l}", f"br{l}"):
                        consts[nm] = load_const(nm)
                if has_gbias[l - 1]:
                    consts[f"biasb{l}"] = load_const(f"biasb{l}")
            for nm in ("Wc1t", "Wc1b", "Wc2"):
                consts[nm] = load_const(nm, dtype=BF16, chunked=True)
            consts["Wc3"] = load_const("Wc3", dtype=BF16)
            if has_bc1:
                consts["bc1"] = load_const("bc1")
            for nm in ("bc2c", "nbc2c", "bc3c", "ones", "identf"):
                consts[nm] = load_const(nm)
            iotar = load_const("iotar", dtype=BF16)
            ones = consts["ones"]
            identf = consts["identf"]

            xr_w = cp.tile([128, wpc, HID], BF16, tag="xr_w")
            hbuf = cp.tile([128, wpc, HID], F32, tag="hbuf")
            b_win = cp.tile([128, wpc, HID], BF16, tag="b_win")

            # DRAM scratch
            xl_in = {
                l: dp.tile([L, HID], BF16, tag=f"xl_in{l}", name=f"xl_in{l}")
                for l in (1, 2, 3)
            }
            xl_full = {
                l: dp.tile([S, HID], BF16, tag=f"xl_full{l}",
                           name=f"xl_full{l}", addr_space="Shared")
                for l in (1, 2, 3)
            }
            a_in = dp.tile([L, HID], BF16, tag="a_in")
            a_full = dp.tile([S, HID], BF16, tag="a_full",
                             addr_space="Shared")

            def transposes(w):
                """hbuf[:, w, :] (f32) -> xfm chunks (bf16, transposed)."""
                ws = slice(w * 128, (w + 1) * 128)
                for c in range(2):
                    tp = psn.tile([128, 512], F32, tag="psm")
                    nc.tensor.transpose(
                        tp[:, :128], hbuf[:, w, c * 128 : (c + 1) * 128], identf[:]
                    )
                    nc.scalar.activation(xfm[c][:, ws], tp[:, :128], ACT.Copy)

            def node(l, w):
                """xl (-> DRAM) and xr (-> SBUF) node transforms for layer l."""
                ws = slice(w * 128, (w + 1) * 128)
                nk = nin if l == 1 else 2
                emit_bias = has_nbias[l - 1]
                for side, Wn, bn in (
                    ("l", f"Wl{l}", f"bl{l}"),
                    ("r", f"Wr{l}", f"br{l}"),
                ):
                    ps = psn.tile([128, 512], F32, tag="psm")
                    pz = ps[:, :HID]
                    for c in range(nk):
                        nc.tensor.matmul(
                            pz,
                            lhsT=xfm[c][:, ws],
                            rhs=consts[Wn][:, c, :],
                            start=(c == 0),
                            stop=(not emit_bias and c == nk - 1),
                        )
                    if emit_bias:
                        nc.tensor.matmul(
                            pz,
                            lhsT=ones[:1, 0:128],
                            rhs=consts[bn][:1, :],
                            start=False,
                            stop=True,
                        )
                    if side == "l":
                        xo = wp.tile([128, HID], BF16, tag="xo")
                        nc.scalar.activation(xo[:], pz, ACT.Copy)
                        nc.sync.dma_start(xl_in[l][ws, :], xo[:])
                    else:
                        nc.scalar.activation(xr_w[:, w, :], pz, ACT.Copy)

            def node_ab(w):
                """a = h3 @ Wc1[:256] (-> DRAM), b = h3 @ Wc1[256:] + bc1."""
                ws = slice(w * 128, (w + 1) * 128)
                ps = psn.tile([128, 512], F32, tag="psm")
                pa = ps[:, :HID]
                for c in range(2):
                    nc.tensor.matmul(
                        pa, lhsT=xfm[c][:, ws],
                        rhs=consts["Wc1t"][:, c, :],
                        start=(c == 0), stop=(c == 1),
                    )
                xo = wp.tile([128, HID], BF16, tag="xo")
                nc.scalar.activation(xo[:], pa, ACT.Copy)
                nc.sync.dma_start(a_in[ws, :], xo[:])
                ps2 = psn.tile([128, 512], F32, tag="psm")
                pb = ps2[:, :HID]
                for c in range(2):
                    nc.tensor.matmul(
                        pb, lhsT=xfm[c][:, ws],
                        rhs=consts["Wc1b"][:, c, :],
                        start=(c == 0), stop=(not has_bc1 and c == 1),
                    )
                if has_bc1:
                    nc.tensor.matmul(
                        pb, lhsT=ones[:1, 0:128],
                        rhs=consts["bc1"][:1, :],
                        start=False, stop=True,
                    )
                nc.scalar.activation(b_win[:, w, :], pb, ACT.Copy)

            def edge(l, w):
                """GATv2 edge phase for window w of layer l -> hbuf[:, w, :]."""
                gt = ep.tile([128, T, HID], BF16, tag="gt")
                nc.gpsimd.dma_gather(
                    out_ap=gt[:],
                    in_ap=xl_full[l][:],
                    idxs_ap=esrc_sb[:, w, :],
                    num_idxs=ew,
                    num_idxs_reg=ew,
                    elem_size=HID,
                    single_packet=False,
                )
                # one-hot st[p, t, n] = (n == doff[p, t]) for the scatter
                st = ep.tile([128, T, 128], BF16, tag="st")
                nc.vector.tensor_tensor(
                    out=st[:],
                    in0=iotar[:].unsqueeze(1).to_broadcast([128, T, 128]),
                    in1=doff_sb[:, w, :].unsqueeze(2).to_broadcast([128, T, 128]),
                    op=ALU.is_equal,
                )
                # m = gt + xr[dst]; xr[dst] via stT one-hot matmuls (pairs);
                # scalar engine evacuates psum -> bf16, one big DVE add after
                m = ep.tile([128, T, HID], BF16, tag="m")
                rt = ep.tile([128, T, HID], BF16, tag="rt")
                for tp in range(0, T, 2):
                    k = min(2, T - tp)
                    ps = psn.tile([128, 512], F32, tag="psm")
                    for i in range(k):
                        nc.tensor.matmul(
                            ps[:, i * HID : (i + 1) * HID],
                            lhsT=stT_sb[:, w, tp + i, :],
                            rhs=xr_w[:, w, :],
                            start=True,
                            stop=True,
                        )
                    nc.scalar.activation(
                        rt[:, tp : tp + k, :].rearrange("p k f -> p (k f)"),
                        ps[:, : k * HID], ACT.Copy,
                    )
                nc.vector.tensor_tensor(
                    out=m[:], in0=gt[:], in1=rt[:], op=ALU.add
                )
                # leaky relu: max(x, 0.2x) on DVE (wx[:, :, :HID] as scratch)
                wx = ep.tile([128, T, HID + H], BF16, tag="wx")
                lk = wx[:, :, 0:HID]
                nc.vector.tensor_scalar(
                    out=lk, in0=m[:], scalar1=0.2, scalar2=None, op0=ALU.mult
                )
                nc.vector.tensor_tensor(out=m[:], in0=m[:], in1=lk, op=ALU.max)
                nc.vector.tensor_tensor(
                    out=m[:],
                    in0=m[:],
                    in1=consts[f"attb{l}"][:].unsqueeze(1).to_broadcast(
                        [128, T, HID]
                    ),
                    op=ALU.mult,
                )
                lg = ep.tile([128, T, H], F32, tag="lg")
                nc.vector.tensor_reduce(
                    out=lg[:].rearrange("p t h -> p (t h)"),
                    in_=m[:].rearrange("p t (g c) -> p (t g) c", c=C),
                    axis=mybir.AxisListType.X,
                    op=ALU.add,
                )
                nc.scalar.activation(wx[:, :, HID : HID + H], lg[:], ACT.Exp)
                nc.vector.tensor_tensor(
                    out=wx[:, :, 0:HID].rearrange("p t (h c) -> p t h c", c=C),
                    in0=gt[:].rearrange("p t (h c) -> p t h c", c=C),
                    in1=wx[:, :, HID : HID + H]
                    .unsqueeze(3)
                    .to_broadcast([128, T, H, C]),
                    op=ALU.mult,
                )
                ops = pso.tile([128, HID + H], F32, tag="ops")
                for t in range(T):
                    nc.tensor.matmul(
                        ops[:],
                        lhsT=st[:, t, :],
                        rhs=wx[:, t, :],
                        start=(t == 0),
                        stop=(t == T - 1),
                    )
                rc = ep.tile([128, H], F32, tag="rc")
                nc.vector.reciprocal(rc[:], ops[:, HID : HID + H])
                nc.vector.tensor_tensor(
                    out=hbuf[:, w, :].rearrange("p (h c) -> p h c", c=C),
                    in0=ops[:, 0:HID].rearrange("p (h c) -> p h c", c=C),
                    in1=rc[:].unsqueeze(2).to_broadcast([128, H, C]),
                    op=ALU.mult,
                )
                if has_gbias[l - 1]:
                    nc.vector.tensor_tensor(
                        out=hbuf[:, w, :],
                        in0=hbuf[:, w, :],
                        in1=consts[f"biasb{l}"][:],
                        op=ALU.add,
                    )
                if l <= 2:
                    # true ELU: exp(min(h,0)) - 1 + max(h,0) (no +1 offset --
                    # bf16 storage keeps full mantissa on the signal)
                    te = ep1.tile([128, HID], F32, tag="te")
                    nc.scalar.activation(
                        te[:], hbuf[:, w, :], ACT.Relu, scale=-1.0
                    )
                    nc.scalar.activation(te[:], te[:], ACT.Exp, scale=-1.0)
                    nc.vector.tensor_scalar(
                        out=te[:], in0=te[:], scalar1=-1.0, scalar2=None,
                        op0=ALU.add,
                    )
                    nc.scalar.activation(
                        hbuf[:, w, :], hbuf[:, w, :], ACT.Relu
                    )
                    nc.vector.tensor_add(hbuf[:, w, :], hbuf[:, w, :], te[:])

            def cls(w):
                """Edge classifier over window w's edge slots (feature-major)."""
                agT = ep.tile([128, HID // 128, ew], BF16, tag="agT")
                nc.gpsimd.dma_gather(
                    out_ap=agT[:],
                    in_ap=a_full[:],
                    idxs_ap=esrc_sb[:, w, :],
                    num_idxs=ew,
                    num_idxs_reg=ew,
                    elem_size=HID,
                    transpose=True,
                    single_packet=False,
                )
                z1 = ep1.tile([128, 2, ew], BF16, tag="z1")
                for mh in range(2):
                    for ch in range(NEC):
                        ps1 = psc.tile([128, EC], F32, tag="psc")
                        for i in range(EC // 128):
                            t = ch * (EC // 128) + i
                            nc.tensor.matmul(
                                ps1[:, i * 128 : (i + 1) * 128],
                                lhsT=b_win[:, w, mh * 128 : (mh + 1) * 128],
                                rhs=stT_sb[:, w, t, :],
                                start=True,
                                stop=True,
                            )
                        es = slice(ch * EC, (ch + 1) * EC)
                        sl = z1[:, mh, es]
                        nc.vector.tensor_tensor(
                            out=sl, in0=agT[:, mh, es], in1=ps1[:], op=ALU.add
                        )
                        # true ELU: (exp(min)-1) + max
                        t2 = ep1.tile([128, EC], F32, tag="t2")
                        nc.vector.tensor_scalar(
                            out=t2[:], in0=sl, scalar1=0.0, scalar2=None,
                            op0=ALU.min,
                        )
                        nc.scalar.activation(t2[:], t2[:], ACT.Exp)
                        nc.vector.tensor_scalar(
                            out=t2[:], in0=t2[:], scalar1=-1.0, scalar2=None,
                            op0=ALU.add,
                        )
                        nc.vector.tensor_scalar(
                            out=sl, in0=sl, scalar1=0.0, scalar2=None,
                            op0=ALU.max,
                        )
                        nc.vector.tensor_add(sl, sl, t2[:])
                z2 = ep1.tile([128, ew], BF16, tag="z2")
                for ch in range(NEC):
                    es = slice(ch * EC, (ch + 1) * EC)
                    ps2 = psc.tile([128, EC], F32, tag="psc")
                    for mh in range(2):
                        nc.tensor.matmul(
                            ps2[:],
                            lhsT=consts["Wc2"][:, mh, :],
                            rhs=z1[:, mh, es],
                            start=(mh == 0),
                            stop=(mh == 1),
                        )
                    t2 = ep1.tile([128, EC], F32, tag="t2b")
                    nc.scalar.activation(
                        t2[:], ps2[:], ACT.Relu, scale=-1.0,
                        bias=consts["nbc2c"][:, 0:1],
                    )
                    nc.scalar.activation(t2[:], t2[:], ACT.Exp, scale=-1.0)
                    nc.vector.tensor_scalar(
                        out=t2[:], in0=t2[:], scalar1=-1.0, scalar2=None,
                        op0=ALU.add,
                    )
                    nc.scalar.activation(
                        z2[:, es], ps2[:], ACT.Relu, bias=consts["bc2c"][:, 0:1]
                    )
                    nc.vector.tensor_add(z2[:, es], z2[:, es], t2[:])
                zo = ep1.tile([NCLS, ew], F32, tag="zo")
                for ch in range(NEC):
                    es = slice(ch * EC, (ch + 1) * EC)
                    ps3 = psc.tile([128, EC], F32, tag="psc")
                    nc.tensor.matmul(
                        ps3[:NCLS, :], lhsT=consts["Wc3"][:],
                        rhs=z2[:, es],
                        start=True, stop=True,
                    )
                    nc.scalar.activation(
                        zo[:, es], ps3[:NCLS, :], ACT.Identity,
                        bias=consts["bc3c"][:, 0:1],
                    )
                nc.sync.dma_start(out_t[:, w * ew : (w + 1) * ew], zo[:])

            # ================= schedule
            def ag(src_t, dst_t):
                nc.gpsimd.collective_compute(
                    "AllGather",
                    ALU.bypass,
                    replica_groups=rg,
                    ins=[src_t[:].opt()],
                    outs=[dst_t[:].opt()],
                )

            for w in range(wpc):
                node(1, w)
            ag(xl_in[1], xl_full[1])
            for l in (1, 2, 3):
                for w in range(wpc):
                    edge(l, w)
                    transposes(w)
                    if l < 3:
                        node(l + 1, w)
                    else:
                        node_ab(w)
                if l < 3:
                    ag(xl_in[l + 1], xl_full[l + 1])
                else:
                    ag(a_in, a_full)
            for w in range(wpc):
                cls(w)

    nc.compile()
    return nc


# ---------------------------------------------------------------- entry point

_CACHE = {}


def run(inputs, cfg, **kw):
    in_maps, meta = prepare_host(inputs, cfg)
    key = (cfg.n, cfg.e, cfg.wpc, cfg.T, cfg.in_dim, meta["flags"])
    if key not in _CACHE:
        _CACHE[key] = build_nc(cfg, flags=meta["flags"])
    nc = _CACHE[key]
    res = run_bass_kernel_spmd(nc, in_maps, list(range(NCORES)), **kw)
    out = np.zeros((cfg.e, NCLS), np.float32)
    for c in range(NCORES):
        o = np.asarray(res.results[c]["out_t"], np.float32)  # [5, wpc*ew]
        pos, orig = meta["out_maps"][c]
        out[orig] = o[:, pos].T
    return out, res


def kernel(**inputs) -> np.ndarray:
    n = inputs["x"].shape[0]
    e = inputs["edge_index"].shape[1]
    wpc = -(-n // (NCORES * 128))
    cfg = Cfg(n, e, wpc=wpc, T=9, in_dim=inputs["x"].shape[1])
    while True:
        try:
            out, _ = run(inputs, cfg)
            return out
        except AssertionError as ex:
            if "window overflow" in str(ex) and cfg.T < 16:
                cfg = Cfg(n, e, wpc=wpc, T=cfg.T + 3, in_dim=inputs["x"].shape[1])
                continue
            raise


# revision 20
# speedup vs baseline: 1.0325x; 1.0325x over previous
"""Trainium2 Bass kernel for nn_ScoreGraphReconstructor (3-layer GATv2 + edge MLP).

Sharding: nodes are permuted into 8*WPC windows of 128 slots, balanced by
in-degree. Each core owns WPC windows (contiguous slot range) and all edges
whose *target* lands in its windows, so the segment softmax/scatter is fully
core-local. Per layer: node matmuls on the local shard -> AllGather of the
source-side transform xl (bf16) -> edge phase (one dma_gather of xl[src],
xr[dst] reconstructed via one-hot transpose matmuls from SBUF, attention on
DVE/ACT, one-hot scatter matmul into PSUM). The edge classifier rides the same
window layout: per-node a/b = h3 @ Wc1 halves, AllGather a, transposed gather
of a[src] with the same indices, b[dst] via the same one-hots; z2/z3 are
feature-major matmuls. All matmuls bf16 (fp32 is 4x slower on the PE).
"""

import sys

for _p in ("/opt/trn_rl_repo",):
    if _p not in sys.path:
        sys.path.insert(0, _p)

import numpy as np
from ml_dtypes import bfloat16 as np_bf16

import concourse.bass as bass
import concourse.bacc as bacc
import concourse.mybir as mybir
import concourse.tile as tile
from concourse.bass_utils import run_bass_kernel_spmd

F32 = mybir.dt.float32
F32R = mybir.dt.float32r
BF16 = mybir.dt.bfloat16
I16 = mybir.dt.int16

NCORES = 8
H, C = 4, 64
HID = H * C  # 256
NCLS = 5
ACT = mybir.ActivationFunctionType
ALU = mybir.AluOpType


class Cfg:
    def __init__(self, n_nodes, n_edges, wpc, T, in_dim=256):
        self.n = n_nodes
        self.e = n_edges
        self.wpc = wpc                    # windows per core
        self.T = T                        # edge tiles (of 128) per window
        self.L = wpc * 128                # local slots per core
        self.S = NCORES * self.L          # total slots
        self.nwin = NCORES * self.wpc
        self.ew = T * 128                 # edge slots per window
        self.in_dim = in_dim


# ---------------------------------------------------------------- host prep


def _balance_windows(deg, nwin):
    """Assign node n (with weight deg[n]) to one of nwin windows, each holding
    exactly 128 nodes (rest dummy), minimizing max window load. Greedy LPT."""
    import heapq

    n = len(deg)
    order = np.argsort(-deg, kind="stable")
    heap = [(0, w) for w in range(nwin)]
    heapq.heapify(heap)
    count = np.zeros(nwin, np.int64)
    slot_of = np.empty(n, np.int64)
    pos = np.zeros(nwin, np.int64)
    for node in order:
        while True:
            load, w = heapq.heappop(heap)
            if count[w] < 128:
                break
        slot_of[node] = w * 128 + pos[w]
        pos[w] += 1
        count[w] += 1
        if count[w] < 128:
            heapq.heappush(heap, (load + int(deg[node]), w))
    return slot_of


def _wrap_idx16(idx, rows=128):
    """dma_gather index layout: index i lives at [i % 16, i // 16] of a
    (rows, len/16) int16 SBUF tile; rows 16..127 padded with copies."""
    n = len(idx)
    assert n % 16 == 0
    blk = np.asarray(idx, np.int16).reshape(n // 16, 16).T
    return np.tile(blk, (rows // 16, 1))


def prepare_host(inputs, cfg):
    """Build per-core input maps + metadata. inputs: dict from setup_inputs."""
    cn = {k: np.asarray(v) for k, v in inputs.items()}
    x = cn["x"].astype(np.float32)
    ei = cn["edge_index"].astype(np.int64)
    row, col = ei[0], ei[1]
    n, e = cfg.n, cfg.e
    L, S, wpc, T = cfg.L, cfg.S, cfg.wpc, cfg.T
    ew = cfg.ew

    deg = np.bincount(col, minlength=n) + 1  # +1 self loop
    slot_of = _balance_windows(deg, cfg.nwin)

    # --- edge lists (conv graph: original edges + self loops on ALL slots)
    all_slots = np.arange(S, dtype=np.int64)
    src_sl = np.concatenate([slot_of[row], all_slots])
    dst_sl = np.concatenate([slot_of[col], all_slots])
    orig_id = np.concatenate(
        [np.arange(e, dtype=np.int64), np.full(S, -1, np.int64)]
    )
    win = dst_sl // 128
    ordr = np.argsort(win, kind="stable")
    src_sl, dst_sl, win, orig_id = (
        src_sl[ordr], dst_sl[ordr], win[ordr], orig_id[ordr]
    )
    starts = np.searchsorted(win, np.arange(cfg.nwin))
    ends = np.searchsorted(win, np.arange(cfg.nwin), side="right")
    maxcnt = int((ends - starts).max())
    assert maxcnt <= ew, f"window overflow: {maxcnt} > {ew}; raise T"

    # per-core edge tensors + output maps
    per_core = []
    out_maps = []
    for c in range(NCORES):
        esrc = np.zeros((128, wpc, ew // 16), np.int16)
        doff = np.full((128, wpc, T), 128, np.float32)
        poss, origs = [], []
        for wi in range(wpc):
            w = c * wpc + wi
            s0, s1 = starts[w], ends[w]
            cnt = s1 - s0
            srcw = np.zeros(ew, np.int64)
            dofw = np.full(ew, 128, np.int64)
            srcw[:cnt] = src_sl[s0:s1]
            dofw[:cnt] = dst_sl[s0:s1] - w * 128
            esrc[:, wi, :] = _wrap_idx16(srcw)
            # edge j -> partition j%128, tile j//128 (both gather + doff)
            doff[:, wi, :] = dofw.reshape(T, 128).T
            oid = orig_id[s0:s1]
            real = oid >= 0
            poss.append(wi * ew + np.nonzero(real)[0])
            origs.append(oid[real])
        # one-hot transposed: stT[nslot, wi, t, p] = (doff[p, wi, t] == nslot)
        stT = (
            np.arange(128, dtype=np.int32)[:, None, None, None]
            == doff[None].transpose(0, 2, 3, 1)
        ).astype(np_bf16)
        per_core.append(
            {
                "esrc": esrc,
                "doff": doff.astype(np_bf16),
                "stT": stT,
            }
        )
        out_maps.append(
            (np.concatenate(poss), np.concatenate(origs))
        )

    # --- node features, permuted + transposed
    xp = np.zeros((S, cfg.in_dim), np.float32)
    xp[slot_of] = x
    nin = cfg.in_dim // 128

    # --- weights with ELU(-1) folds
    wmaps = {}
    for l in (1, 2, 3):
        kin = cfg.in_dim if l == 1 else HID
        Wl = cn[f"W{l}l"].astype(np.float32)
        Wr = cn[f"W{l}r"].astype(np.float32)
        bl = cn[f"b{l}l"].astype(np.float32).copy()
        br = cn[f"b{l}r"].astype(np.float32).copy()
        att = cn[f"att{l}"].astype(np.float32)
        wmaps[f"Wl{l}"] = Wl.reshape(kin // 128, 128, HID).astype(np_bf16)
        wmaps[f"Wr{l}"] = Wr.reshape(kin // 128, 128, HID).astype(np_bf16)
        wmaps[f"bl{l}"] = bl.reshape(1, HID)
        wmaps[f"br{l}"] = br.reshape(1, HID)
        wmaps[f"attb{l}"] = np.tile(att.reshape(1, HID), (128, 1)).astype(np_bf16)
        wmaps[f"biasb{l}"] = np.tile(
            cn[f"bias{l}"].astype(np.float32).reshape(1, HID), (128, 1)
        )
    Wc1 = cn["Wc1"].astype(np.float32)
    Wc2 = cn["Wc2"].astype(np.float32)
    Wc3 = cn["Wc3"].astype(np.float32)
    wmaps["Wc1t"] = Wc1[:HID].reshape(2, 128, HID).astype(np_bf16)
    wmaps["Wc1b"] = Wc1[HID:].reshape(2, 128, HID).astype(np_bf16)
    wmaps["bc1"] = cn["bc1"].astype(np.float32).reshape(1, HID)
    wmaps["Wc2"] = Wc2.reshape(2, 128, HID // 2).astype(np_bf16)
    wmaps["Wc3"] = Wc3.astype(np_bf16)  # [128, 5]
    bc2f = cn["bc2"].astype(np.float32)
    bc3f = cn["bc3"].astype(np.float32)
    wmaps["bc2c"] = bc2f.reshape(HID // 2, 1)
    wmaps["nbc2c"] = (-bc2f).reshape(HID // 2, 1)
    wmaps["bc3c"] = bc3f.reshape(NCLS, 1)
    wmaps["ones"] = np.ones((1, 512), np.float32)
    wmaps["identf"] = np.eye(128, dtype=np.float32)
    wmaps["iotar"] = np.tile(
        np.arange(128, dtype=np.float32), (128, 1)
    ).astype(np_bf16)

    in_maps = []
    for c in range(NCORES):
        m = dict(wmaps)
        m.update(per_core[c])
        xc = xp[c * L : (c + 1) * L].T.copy()  # (in_dim, L)
        m["xfm"] = xc.reshape(nin, 128, L).astype(np_bf16)
        in_maps.append(m)

    flags = (
        tuple(bool(np.any(cn[f"b{l}l"]) or np.any(cn[f"b{l}r"])) for l in (1, 2, 3)),
        tuple(bool(np.any(cn[f"bias{l}"])) for l in (1, 2, 3)),
        bool(np.any(cn["bc1"])),
    )
    meta = {"slot_of": slot_of, "cfg": cfg, "out_maps": out_maps,
            "flags": flags}
    return in_maps, meta


# ---------------------------------------------------------------- device build


def build_nc(cfg, flags=((True,) * 3, (True,) * 3, True), debug=False):
    nc = bacc.Bacc("TRN2", target_bir_lowering=False, debug=debug, num_devices=NCORES)
    has_nbias, has_gbias, has_bc1 = flags
    L, S, wpc, T, ew = cfg.L, cfg.S, cfg.wpc, cfg.T, cfg.ew
    nin = cfg.in_dim // 128
    EC = 384                  # classifier edge-chunk (z1/z2/z3 psum width)
    NEC = ew // EC
    assert ew % EC == 0 and EC % 128 == 0

    P = {}

    def pin(name, shape, dtype=F32):
        P[name] = nc.declare_dram_parameter(name, list(shape), dtype, isOutput=False)

    pin("xfm", (nin, 128, L), BF16)
    pin("esrc", (128, wpc, ew // 16), I16)
    pin("doff", (128, wpc, T), BF16)
    pin("stT", (128, wpc, T, 128), BF16)
    for l in (1, 2, 3):
        nk = nin if l == 1 else 2
        pin(f"Wl{l}", (nk, 128, HID), BF16)
        pin(f"Wr{l}", (nk, 128, HID), BF16)
        pin(f"bl{l}", (1, HID))
        pin(f"br{l}", (1, HID))
        pin(f"attb{l}", (128, HID), BF16)
        pin(f"biasb{l}", (128, HID))
    pin("Wc1t", (2, 128, HID), BF16)
    pin("Wc1b", (2, 128, HID), BF16)
    pin("bc1", (1, HID))
    pin("Wc2", (2, 128, HID // 2), BF16)
    pin("Wc3", (128, NCLS), BF16)
    pin("bc2c", (HID // 2, 1))
    pin("nbc2c", (HID // 2, 1))
    pin("bc3c", (NCLS, 1))
    pin("ones", (1, 512))
    pin("identf", (128, 128))
    pin("iotar", (128, 128), BF16)
    out_t = nc.declare_dram_parameter(
        "out_t", [NCLS, wpc * ew], F32, isOutput=True
    )

    rg = [list(range(NCORES))]

    with tile.TileContext(nc) as tc:
        with (
            tc.tile_pool(name="const", bufs=1) as cp,
            tc.tile_pool(name="dram", bufs=1, space="DRAM") as dp,
            tc.tile_pool(name="work", bufs=2) as wp,
            tc.tile_pool(name="epbig", bufs=3) as ep,
            tc.tile_pool(name="ep1", bufs=1) as ep1,
            tc.tile_pool(name="psn", bufs=2, space="PSUM") as psn,
            tc.tile_pool(name="pso", bufs=2, space="PSUM") as pso,
            tc.tile_pool(name="psc", bufs=2, space="PSUM") as psc,
        ):
            # ---------- constants into SBUF
            def load_const(name, dtype=F32, chunked=False):
                src = P[name]
                shp = list(src.shape)
                if chunked:
                    # [nk, 128, D] DRAM -> [128, nk, D] SBUF (weight chunks)
                    t = cp.tile([shp[1], shp[0], shp[2]], dtype, tag=name,
                                name=name + "_sb")
                    for c in range(shp[0]):
                        nc.sync.dma_start(t[:, c, :], src[c])
                else:
                    t = cp.tile(shp, dtype, tag=name, name=name + "_sb")
                    nc.sync.dma_start(t[:], src[:])
                return t

            xfm = [
                cp.tile([128, L], BF16, tag=f"xfm{c}", name=f"xfm{c}")
                for c in range(nin)
            ]
            for c in range(nin):
                nc.sync.dma_start(xfm[c][:], P["xfm"][c])
            esrc_sb = load_const("esrc", dtype=I16)
            doff_sb = load_const("doff", dtype=BF16)
            stT_sb = load_const("stT", dtype=BF16)
            consts = {}
            for l in (1, 2, 3):
                for nm in (f"Wl{l}", f"Wr{l}"):
                    consts[nm] = load_const(nm, dtype=BF16, chunked=True)
                consts[f"attb{l}"] = load_const(f"attb{l}", dtype=BF16)
                if has_nbias[l - 1]:
                    for nm in (f"bl{l}", f"br{l}"):
                        consts[nm] = load_const(nm)
                if has_gbias[l - 1]:
                    consts[f"biasb{l}"] = load_const(f"biasb{l}")
            for nm in ("Wc1t", "Wc1b", "Wc2"):
                consts[nm] = load_const(nm, dtype=BF16, chunked=True)
            consts["Wc3"] = load_const("Wc3", dtype=BF16)
            if has_bc1:
                consts["bc1"] = load_const("bc1")
            for nm in ("bc2c", "nbc2c", "bc3c", "ones", "identf"):
                consts[nm] = load_const(nm)
            iotar = load_const("iotar", dtype=BF16)
            ones = consts["ones"]
            identf = consts["identf"]

            xr_w = cp.tile([128, wpc, HID], BF16, tag="xr_w")
            hbuf = cp.tile([128, wpc, HID], F32, tag="hbuf")
            b_win = cp.tile([128, wpc, HID], BF16, tag="b_win")

            # DRAM scratch
            xl_in = {
                l: dp.tile([L, HID], BF16, tag=f"xl_in{l}", name=f"xl_in{l}")
                for l in (1, 2, 3)
            }
            xl_full = {
                l: dp.tile([S, HID], BF16, tag=f"xl_full{l}",
                           name=f"xl_full{l}", addr_space="Shared")
                for l in (1, 2, 3)
            }
            a_in = dp.tile([L, HID], BF16, tag="a_in")
            a_full = dp.tile([S, HID], BF16, tag="a_full",
                             addr_space="Shared")

            def transposes(w):
                """hbuf[:, w, :] (f32) -> xfm chunks (bf16, transposed)."""
                ws = slice(w * 128, (w + 1) * 128)
                for c in range(2):
                    tp = psn.tile([128, 512], F32, tag="psm")
                    nc.tensor.transpose(
                        tp[:, :128], hbuf[:, w, c * 128 : (c + 1) * 128], identf[:]
                    )
                    nc.scalar.activation(xfm[c][:, ws], tp[:, :128], ACT.Copy)

            def node(l, w):
                """xl (-> DRAM) and xr (-> SBUF) node transforms for layer l."""
                ws = slice(w * 128, (w + 1) * 128)
                nk = nin if l == 1 else 2
                emit_bias = has_nbias[l - 1]
                for side, Wn, bn in (
                    ("l", f"Wl{l}", f"bl{l}"),
                    ("r", f"Wr{l}", f"br{l}"),
                ):
                    ps = psn.tile([128, 512], F32, tag="psm")
                    pz = ps[:, :HID]
                    for c in range(nk):
                        nc.tensor.matmul(
                            pz,
                            lhsT=xfm[c][:, ws],
                            rhs=consts[Wn][:, c, :],
                            start=(c == 0),
                            stop=(not emit_bias and c == nk - 1),
                        )
                    if emit_bias:
                        nc.tensor.matmul(
                            pz,
                            lhsT=ones[:1, 0:128],
                            rhs=consts[bn][:1, :],
                            start=False,
                            stop=True,
                        )
                    if side == "l":
                        xo = wp.tile([128, HID], BF16, tag="xo")
                        nc.scalar.activation(xo[:], pz, ACT.Copy)
                        nc.sync.dma_start(xl_in[l][ws, :], xo[:])
                    else:
                        nc.scalar.activation(xr_w[:, w, :], pz, ACT.Copy)

            def node_ab(w):
                """a = h3 @ Wc1[:256] (-> DRAM), b = h3 @ Wc1[256:] + bc1."""
                ws = slice(w * 128, (w + 1) * 128)
                ps = psn.tile([128, 512], F32, tag="psm")
                pa = ps[:, :HID]
                for c in range(2):
                    nc.tensor.matmul(
                        pa, lhsT=xfm[c][:, ws],
                        rhs=consts["Wc1t"][:, c, :],
                        start=(c == 0), stop=(c == 1),
                    )
                xo = wp.tile([128, HID], BF16, tag="xo")
                nc.scalar.activation(xo[:], pa, ACT.Copy)
                nc.sync.dma_start(a_in[ws, :], xo[:])
                ps2 = psn.tile([128, 512], F32, tag="psm")
                pb = ps2[:, :HID]
                for c in range(2):
                    nc.tensor.matmul(
                        pb, lhsT=xfm[c][:, ws],
                        rhs=consts["Wc1b"][:, c, :],
                        start=(c == 0), stop=(not has_bc1 and c == 1),
                    )
                if has_bc1:
                    nc.tensor.matmul(
                        pb, lhsT=ones[:1, 0:128],
                        rhs=consts["bc1"][:1, :],
                        start=False, stop=True,
                    )
                nc.scalar.activation(b_win[:, w, :], pb, ACT.Copy)

            def edge(l, w):
                """GATv2 edge phase for window w of layer l -> hbuf[:, w, :]."""
                gt = ep.tile([128, T, HID], BF16, tag="gt")
                nc.gpsimd.dma_gather(
                    out_ap=gt[:],
                    in_ap=xl_full[l][:],
                    idxs_ap=esrc_sb[:, w, :],
                    num_idxs=ew,
                    num_idxs_reg=ew,
                    elem_size=HID,
                    single_packet=False,
                )
                # one-hot st[p, t, n] = (n == doff[p, t]) for the scatter
                st = ep.tile([128, T, 128], BF16, tag="st")
                nc.vector.tensor_tensor(
                    out=st[:],
                    in0=iotar[:].unsqueeze(1).to_broadcast([128, T, 128]),
                    in1=doff_sb[:, w, :].unsqueeze(2).to_broadcast([128, T, 128]),
                    op=ALU.is_equal,
                )
                # m = gt + xr[dst]; xr[dst] via stT one-hot matmuls (pairs)
                m = ep.tile([128, T, HID], BF16, tag="m")
                for tp in range(0, T, 2):
                    k = min(2, T - tp)
                    ps = psn.tile([128, 512], F32, tag="psm")
                    for i in range(k):
                        nc.tensor.matmul(
                            ps[:, i * HID : (i + 1) * HID],
                            lhsT=stT_sb[:, w, tp + i, :],
                            rhs=xr_w[:, w, :],
                            start=True,
                            stop=True,
                        )
                    nc.vector.tensor_tensor(
                        out=m[:, tp : tp + k, :],
                        in0=gt[:, tp : tp + k, :],
                        in1=ps[:, : k * HID].rearrange("p (k f) -> p k f", f=HID),
                        op=ALU.add,
                    )
                # leaky relu: max(x, 0.2x) on DVE (wx[:, :, :HID] as scratch)
                wx = ep.tile([128, T, HID + H], BF16, tag="wx")
                lk = wx[:, :, 0:HID]
                nc.vector.tensor_scalar(
                    out=lk, in0=m[:], scalar1=0.2, scalar2=None, op0=ALU.mult
                )
                nc.vector.tensor_tensor(out=m[:], in0=m[:], in1=lk, op=ALU.max)
                nc.vector.tensor_tensor(
                    out=m[:],
                    in0=m[:],
                    in1=consts[f"attb{l}"][:].unsqueeze(1).to_broadcast(
                        [128, T, HID]
                    ),
                    op=ALU.mult,
                )
                lg = ep.tile([128, T, H], F32, tag="lg")
                nc.vector.tensor_reduce(
                    out=lg[:].rearrange("p t h -> p (t h)"),
                    in_=m[:].rearrange("p t (g c) -> p (t g) c", c=C),
                    axis=mybir.AxisListType.X,
                    op=ALU.add,
                )
                nc.scalar.activation(wx[:, :, HID : HID + H], lg[:], ACT.Exp)
                nc.vector.tensor_tensor(
                    out=wx[:, :, 0:HID].rearrange("p t (h c) -> p t h c", c=C),
                    in0=gt[:].rearrange("p t (h c) -> p t h c", c=C),
                    in1=wx[:, :, HID : HID + H]
                    .unsqueeze(3)
                    .to_broadcast([128, T, H, C]),
                    op=ALU.mult,
                )
                ops = pso.tile([128, HID + H], F32, tag="ops")
                for t in range(T):
                    nc.tensor.matmul(
                        ops[:],
                        lhsT=st[:, t, :],
                        rhs=wx[:, t, :],
                        start=(t == 0),
                        stop=(t == T - 1),
                    )
                rc = ep.tile([128, H], F32, tag="rc")
                nc.vector.reciprocal(rc[:], ops[:, HID : HID + H])
                nc.vector.tensor_tensor(
                    out=hbuf[:, w, :].rearrange("p (h c) -> p h c", c=C),
                    in0=ops[:, 0:HID].rearrange("p (h c) -> p h c", c=C),
                    in1=rc[:].unsqueeze(2).to_broadcast([128, H, C]),
                    op=ALU.mult,
                )
                if has_gbias[l - 1]:
                    nc.vector.tensor_tensor(
                        out=hbuf[:, w, :],
                        in0=hbuf[:, w, :],
                        in1=consts[f"biasb{l}"][:],
                        op=ALU.add,
                    )
                if l <= 2:
                    # true ELU: exp(min(h,0)) - 1 + max(h,0) (no +1 offset --
                    # bf16 storage keeps full mantissa on the signal)
                    te = ep1.tile([128, HID], F32, tag="te")
                    nc.scalar.activation(
                        te[:], hbuf[:, w, :], ACT.Relu, scale=-1.0
                    )
                    nc.scalar.activation(te[:], te[:], ACT.Exp, scale=-1.0)
                    nc.vector.tensor_scalar(
                        out=te[:], in0=te[:], scalar1=-1.0, scalar2=None,
                        op0=ALU.add,
                    )
                    nc.scalar.activation(
                        hbuf[:, w, :], hbuf[:, w, :], ACT.Relu
                    )
                    nc.vector.tensor_add(hbuf[:, w, :], hbuf[:, w, :], te[:])

            def cls(w):
                """Edge classifier over window w's edge slots (feature-major)."""
                agT = ep.tile([128, HID // 128, ew], BF16, tag="agT")
                nc.gpsimd.dma_gather(
                    out_ap=agT[:],
                    in_ap=a_full[:],
                    idxs_ap=esrc_sb[:, w, :],
                    num_idxs=ew,
                    num_idxs_reg=ew,
                    elem_size=HID,
                    transpose=True,
                    single_packet=False,
                )
                z1 = ep1.tile([128, 2, ew], BF16, tag="z1")
                for mh in range(2):
                    for ch in range(NEC):
                        ps1 = psc.tile([128, EC], F32, tag="psc")
                        for i in range(EC // 128):
                            t = ch * (EC // 128) + i
                            nc.tensor.matmul(
                                ps1[:, i * 128 : (i + 1) * 128],
                                lhsT=b_win[:, w, mh * 128 : (mh + 1) * 128],
                                rhs=stT_sb[:, w, t, :],
                                start=True,
                                stop=True,
                            )
                        es = slice(ch * EC, (ch + 1) * EC)
                        sl = z1[:, mh, es]
                        nc.vector.tensor_tensor(
                            out=sl, in0=agT[:, mh, es], in1=ps1[:], op=ALU.add
                        )
                        # true ELU: (exp(min)-1) + max
                        t2 = ep1.tile([128, EC], F32, tag="t2")
                        nc.vector.tensor_scalar(
                            out=t2[:], in0=sl, scalar1=0.0, scalar2=None,
                            op0=ALU.min,
                        )
                        nc.scalar.activation(t2[:], t2[:], ACT.Exp)
                        nc.vector.tensor_scalar(
                            out=t2[:], in0=t2[:], scalar1=-1.0, scalar2=None,
                            op0=ALU.add,
                        )
                        nc.vector.tensor_scalar(
                            out=sl, in0=sl, scalar1=0.0, scalar2=None,
                            op0=ALU.max,
                        )
                        nc.vector.tensor_add(sl, sl, t2[:])
                z2 = ep1.tile([128, ew], BF16, tag="z2")
                for ch in range(NEC):
                    es = slice(ch * EC, (ch + 1) * EC)
                    ps2 = psc.tile([128, EC], F32, tag="psc")
                    for mh in range(2):
                        nc.tensor.matmul(
                            ps2[:],
                            lhsT=consts["Wc2"][:, mh, :],
                            rhs=z1[:, mh, es],
                            start=(mh == 0),
                            stop=(mh == 1),
                        )
                    t2 = ep1.tile([128, EC], F32, tag="t2b")
                    nc.scalar.activation(
                        t2[:], ps2[:], ACT.Relu, scale=-1.0,
                        bias=consts["nbc2c"][:, 0:1],
                    )
                    nc.scalar.activation(t2[:], t2[:], ACT.Exp, scale=-1.0)
                    nc.vector.tensor_scalar(
                        out=t2[:], in0=t2[:], scalar1=-1.0, scalar2=None,
                        op0=ALU.add,
                    )
                    nc.scalar.activation(
                        z2[:, es], ps2[:], ACT.Relu, bias=consts["bc2c"][:, 0:1]
                    )
                    nc.vector.tensor_add(z2[:, es], z2[:, es], t2[:])
                zo = ep1.tile([NCLS, ew], F32, tag="zo")
                for ch in range(NEC):
                    es = slice(ch * EC, (ch + 1) * EC)
                    ps3 = psc.tile([128, EC], F32, tag="psc")
                    nc.tensor.matmul(
                        ps3[:NCLS, :], lhsT=consts["Wc3"][:],
                        rhs=z2[:, es],
                        start=True, stop=True,
                    )
                    nc.scalar.activation(
                        zo[:, es], ps3[:NCLS, :], ACT.Identity,
                        bias=consts["bc3c"][:, 0:1],
                    )
                nc.sync.dma_start(out_t[:, w * ew : (w + 1) * ew], zo[:])

            # ================= schedule
            def ag(src_t, dst_t):
                nc.gpsimd.collective_compute(
                    "AllGather",
                    ALU.bypass,
                    replica_groups=rg,
                    ins=[src_t[:].opt()],
                    outs=[dst_t[:].opt()],
                )

            for w in range(wpc):
                node(1, w)
            ag(xl_in[1], xl_full[1])
            for l in (1, 2, 3):
                for w in range(wpc):
                    edge(l, w)
                    transposes(w)
                    if l < 3:
                        node(l + 1, w)
                    else:
                        node_ab(w)
                if l < 3:
                    ag(xl_in[l + 1], xl_full[l + 1])
                else:
                    ag(a_in, a_full)
            for w in range(wpc):
                cls(w)

    nc.compile()
    return nc


# ---------------------------------------------------------------- entry point

_CACHE = {}


def run(inputs, cfg, **kw):
    in_maps, meta = prepare_host(inputs, cfg)
    key = (cfg.n, cfg.e, cfg.wpc, cfg.T, cfg.in_dim, meta["flags"])
    if key not in _CACHE:
        _CACHE[key] = build_nc(cfg, flags=meta["flags"])
    nc = _CACHE[key]
    res = run_bass_kernel_spmd(nc, in_maps, list(range(NCORES)), **kw)
    out = np.zeros((cfg.e, NCLS), np.float32)
    for c in range(NCORES):
        o = np.asarray(res.results[c]["out_t"], np.float32)  # [5, wpc*ew]
        pos, orig = meta["out_maps"][c]
        out[orig] = o[:, pos].T
    return out, res


def kernel(**inputs) -> np.ndarray:
    n = inputs["x"].shape[0]
    e = inputs["edge_index"].shape[1]
    wpc = -(-n // (NCORES * 128))
    cfg = Cfg(n, e, wpc=wpc, T=9, in_dim=inputs["x"].shape[1])
    while True:
        try:
            out, _ = run(inputs, cfg)
            return out
        except AssertionError as ex:
            if "window overflow" in str(ex) and cfg.T < 16:
                cfg = Cfg(n, e, wpc=wpc, T=cfg.T + 3, in_dim=inputs["x"].shape[1])
                continue
            raise


# revision 22
# speedup vs baseline: 1.0628x; 1.0293x over previous
"""Trainium2 Bass kernel for nn_ScoreGraphReconstructor (3-layer GATv2 + edge MLP).

Sharding: nodes are permuted into 8*WPC windows of 128 slots, balanced by
in-degree. Each core owns WPC windows (contiguous slot range) and all edges
whose *target* lands in its windows, so the segment softmax/scatter is fully
core-local. Per layer: node matmuls on the local shard -> AllGather of the
source-side transform xl (bf16) -> edge phase (one dma_gather of xl[src],
xr[dst] reconstructed via one-hot transpose matmuls from SBUF, attention on
DVE/ACT, one-hot scatter matmul into PSUM). The edge classifier rides the same
window layout: per-node a/b = h3 @ Wc1 halves, AllGather a, transposed gather
of a[src] with the same indices, b[dst] via the same one-hots; z2/z3 are
feature-major matmuls. All matmuls bf16 (fp32 is 4x slower on the PE).
"""

import sys

for _p in ("/opt/trn_rl_repo",):
    if _p not in sys.path:
        sys.path.insert(0, _p)

import numpy as np
from ml_dtypes import bfloat16 as np_bf16

import concourse.bass as bass
import concourse.bacc as bacc
import concourse.mybir as mybir
import concourse.tile as tile
from concourse.bass_utils import run_bass_kernel_spmd

F32 = mybir.dt.float32
F32R = mybir.dt.float32r
BF16 = mybir.dt.bfloat16
I16 = mybir.dt.int16

NCORES = 8
H, C = 4, 64
HID = H * C  # 256
NCLS = 5
ACT = mybir.ActivationFunctionType
ALU = mybir.AluOpType


class Cfg:
    def __init__(self, n_nodes, n_edges, wpc, T, in_dim=256):
        self.n = n_nodes
        self.e = n_edges
        self.wpc = wpc                    # windows per core
        self.T = T                        # edge tiles (of 128) per window
        self.L = wpc * 128                # local slots per core
        self.S = NCORES * self.L          # total slots
        self.nwin = NCORES * self.wpc
        self.ew = T * 128                 # edge slots per window
        self.ewg = (T - 1) * 128          # gathered slots (last tile = self)
        self.in_dim = in_dim


# ---------------------------------------------------------------- host prep


def _balance_windows(deg, nwin):
    """Assign node n (with weight deg[n]) to one of nwin windows, each holding
    exactly 128 nodes (rest dummy), minimizing max window load. Greedy LPT."""
    import heapq

    n = len(deg)
    order = np.argsort(-deg, kind="stable")
    heap = [(0, w) for w in range(nwin)]
    heapq.heapify(heap)
    count = np.zeros(nwin, np.int64)
    slot_of = np.empty(n, np.int64)
    pos = np.zeros(nwin, np.int64)
    for node in order:
        while True:
            load, w = heapq.heappop(heap)
            if count[w] < 128:
                break
        slot_of[node] = w * 128 + pos[w]
        pos[w] += 1
        count[w] += 1
        if count[w] < 128:
            heapq.heappush(heap, (load + int(deg[node]), w))
    return slot_of


def _wrap_idx16(idx, rows=128):
    """dma_gather index layout: index i lives at [i % 16, i // 16] of a
    (rows, len/16) int16 SBUF tile; rows 16..127 padded with copies."""
    n = len(idx)
    assert n % 16 == 0
    blk = np.asarray(idx, np.int16).reshape(n // 16, 16).T
    return np.tile(blk, (rows // 16, 1))


def prepare_host(inputs, cfg):
    """Build per-core input maps + metadata. inputs: dict from setup_inputs."""
    cn = {k: np.asarray(v) for k, v in inputs.items()}
    x = cn["x"].astype(np.float32)
    ei = cn["edge_index"].astype(np.int64)
    row, col = ei[0], ei[1]
    n, e = cfg.n, cfg.e
    L, S, wpc, T = cfg.L, cfg.S, cfg.wpc, cfg.T
    ew = cfg.ew

    deg = np.bincount(col, minlength=n)
    slot_of = _balance_windows(deg, cfg.nwin)

    # --- edge lists (original edges only; self loops ride the synthesized
    # last tile of each window: doff = identity, xl via local DMA)
    src_sl = slot_of[row]
    dst_sl = slot_of[col]
    orig_id = np.arange(e, dtype=np.int64)
    win = dst_sl // 128
    ordr = np.argsort(win, kind="stable")
    src_sl, dst_sl, win, orig_id = (
        src_sl[ordr], dst_sl[ordr], win[ordr], orig_id[ordr]
    )
    starts = np.searchsorted(win, np.arange(cfg.nwin))
    ends = np.searchsorted(win, np.arange(cfg.nwin), side="right")
    maxcnt = int((ends - starts).max())
    assert maxcnt <= cfg.ewg, f"window overflow: {maxcnt} > {cfg.ewg}; raise T"

    # per-core edge tensors + output maps
    per_core = []
    out_maps = []
    ewg = cfg.ewg
    for c in range(NCORES):
        esrc = np.zeros((128, wpc, ewg // 16), np.int16)
        doff = np.full((128, wpc, T), 128, np.float32)
        poss, origs = [], []
        for wi in range(wpc):
            w = c * wpc + wi
            s0, s1 = starts[w], ends[w]
            cnt = s1 - s0
            srcw = np.zeros(ewg, np.int64)
            dofw = np.full(ewg, 128, np.int64)
            srcw[:cnt] = src_sl[s0:s1]
            dofw[:cnt] = dst_sl[s0:s1] - w * 128
            esrc[:, wi, :] = _wrap_idx16(srcw)
            # edge j -> partition j%128, tile j//128 (both gather + doff)
            doff[:, wi, : T - 1] = dofw.reshape(T - 1, 128).T
            doff[:, wi, T - 1] = np.arange(128)  # self-loop tile
            oid = orig_id[s0:s1]
            real = oid >= 0
            poss.append(wi * ewg + np.nonzero(real)[0])
            origs.append(oid[real])
        # one-hot transposed: stT[nslot, wi, t, p] = (doff[p, wi, t] == nslot)
        stT = (
            np.arange(128, dtype=np.int32)[:, None, None, None]
            == doff[None].transpose(0, 2, 3, 1)
        ).astype(np_bf16)
        per_core.append(
            {
                "esrc": esrc,
                "doff": doff.astype(np_bf16),
                "stT": stT,
            }
        )
        out_maps.append(
            (np.concatenate(poss), np.concatenate(origs))
        )

    # --- node features, permuted + transposed
    xp = np.zeros((S, cfg.in_dim), np.float32)
    xp[slot_of] = x
    nin = cfg.in_dim // 128

    # --- weights with ELU(-1) folds
    wmaps = {}
    for l in (1, 2, 3):
        kin = cfg.in_dim if l == 1 else HID
        Wl = cn[f"W{l}l"].astype(np.float32)
        Wr = cn[f"W{l}r"].astype(np.float32)
        bl = cn[f"b{l}l"].astype(np.float32).copy()
        br = cn[f"b{l}r"].astype(np.float32).copy()
        att = cn[f"att{l}"].astype(np.float32)
        wmaps[f"Wl{l}"] = Wl.reshape(kin // 128, 128, HID).astype(np_bf16)
        wmaps[f"Wr{l}"] = Wr.reshape(kin // 128, 128, HID).astype(np_bf16)
        wmaps[f"bl{l}"] = bl.reshape(1, HID)
        wmaps[f"br{l}"] = br.reshape(1, HID)
        wmaps[f"attb{l}"] = np.tile(att.reshape(1, HID), (128, 1)).astype(np_bf16)
        wmaps[f"biasb{l}"] = np.tile(
            cn[f"bias{l}"].astype(np.float32).reshape(1, HID), (128, 1)
        )
    Wc1 = cn["Wc1"].astype(np.float32)
    Wc2 = cn["Wc2"].astype(np.float32)
    Wc3 = cn["Wc3"].astype(np.float32)
    wmaps["Wc1t"] = Wc1[:HID].reshape(2, 128, HID).astype(np_bf16)
    wmaps["Wc1b"] = Wc1[HID:].reshape(2, 128, HID).astype(np_bf16)
    wmaps["bc1"] = cn["bc1"].astype(np.float32).reshape(1, HID)
    wmaps["Wc2"] = Wc2.reshape(2, 128, HID // 2).astype(np_bf16)
    wmaps["Wc3"] = Wc3.astype(np_bf16)  # [128, 5]
    bc2f = cn["bc2"].astype(np.float32)
    bc3f = cn["bc3"].astype(np.float32)
    wmaps["bc2c"] = bc2f.reshape(HID // 2, 1)
    wmaps["nbc2c"] = (-bc2f).reshape(HID // 2, 1)
    wmaps["bc3c"] = bc3f.reshape(NCLS, 1)
    wmaps["ones"] = np.ones((1, 512), np.float32)
    wmaps["identf"] = np.eye(128, dtype=np.float32)
    wmaps["iotar"] = np.tile(
        np.arange(128, dtype=np.float32), (128, 1)
    ).astype(np_bf16)

    in_maps = []
    for c in range(NCORES):
        m = dict(wmaps)
        m.update(per_core[c])
        xc = xp[c * L : (c + 1) * L].T.copy()  # (in_dim, L)
        m["xfm"] = xc.reshape(nin, 128, L).astype(np_bf16)
        in_maps.append(m)

    flags = (
        tuple(bool(np.any(cn[f"b{l}l"]) or np.any(cn[f"b{l}r"])) for l in (1, 2, 3)),
        tuple(bool(np.any(cn[f"bias{l}"])) for l in (1, 2, 3)),
        bool(np.any(cn["bc1"])),
    )
    meta = {"slot_of": slot_of, "cfg": cfg, "out_maps": out_maps,
            "flags": flags}
    return in_maps, meta


# ---------------------------------------------------------------- device build


def build_nc(cfg, flags=((True,) * 3, (True,) * 3, True), debug=False):
    nc = bacc.Bacc("TRN2", target_bir_lowering=False, debug=debug, num_devices=NCORES)
    has_nbias, has_gbias, has_bc1 = flags
    L, S, wpc, T, ew = cfg.L, cfg.S, cfg.wpc, cfg.T, cfg.ew
    ewg = cfg.ewg
    nin = cfg.in_dim // 128
    # classifier edge-chunk (z1/z2/z3 psum width)
    EC = 384 if ewg % 384 == 0 else (256 if ewg % 256 == 0 else 128)
    NEC = ewg // EC

    P = {}

    def pin(name, shape, dtype=F32):
        P[name] = nc.declare_dram_parameter(name, list(shape), dtype, isOutput=False)

    pin("xfm", (nin, 128, L), BF16)
    pin("esrc", (128, wpc, ewg // 16), I16)
    pin("doff", (128, wpc, T), BF16)
    pin("stT", (128, wpc, T, 128), BF16)
    for l in (1, 2, 3):
        nk = nin if l == 1 else 2
        pin(f"Wl{l}", (nk, 128, HID), BF16)
        pin(f"Wr{l}", (nk, 128, HID), BF16)
        pin(f"bl{l}", (1, HID))
        pin(f"br{l}", (1, HID))
        pin(f"attb{l}", (128, HID), BF16)
        pin(f"biasb{l}", (128, HID))
    pin("Wc1t", (2, 128, HID), BF16)
    pin("Wc1b", (2, 128, HID), BF16)
    pin("bc1", (1, HID))
    pin("Wc2", (2, 128, HID // 2), BF16)
    pin("Wc3", (128, NCLS), BF16)
    pin("bc2c", (HID // 2, 1))
    pin("nbc2c", (HID // 2, 1))
    pin("bc3c", (NCLS, 1))
    pin("ones", (1, 512))
    pin("identf", (128, 128))
    pin("iotar", (128, 128), BF16)
    out_t = nc.declare_dram_parameter(
        "out_t", [NCLS, wpc * ewg], F32, isOutput=True
    )

    rg = [list(range(NCORES))]

    with tile.TileContext(nc) as tc:
        with (
            tc.tile_pool(name="const", bufs=1) as cp,
            tc.tile_pool(name="dram", bufs=1, space="DRAM") as dp,
            tc.tile_pool(name="work", bufs=2) as wp,
            tc.tile_pool(name="epbig", bufs=3) as ep,
            tc.tile_pool(name="ep1", bufs=1) as ep1,
            tc.tile_pool(name="psn", bufs=2, space="PSUM") as psn,
            tc.tile_pool(name="pso", bufs=2, space="PSUM") as pso,
            tc.tile_pool(name="psc", bufs=2, space="PSUM") as psc,
        ):
            # ---------- constants into SBUF
            def load_const(name, dtype=F32, chunked=False):
                src = P[name]
                shp = list(src.shape)
                if chunked:
                    # [nk, 128, D] DRAM -> [128, nk, D] SBUF (weight chunks)
                    t = cp.tile([shp[1], shp[0], shp[2]], dtype, tag=name,
                                name=name + "_sb")
                    for c in range(shp[0]):
                        nc.sync.dma_start(t[:, c, :], src[c])
                else:
                    t = cp.tile(shp, dtype, tag=name, name=name + "_sb")
                    nc.sync.dma_start(t[:], src[:])
                return t

            xfm = [
                cp.tile([128, L], BF16, tag=f"xfm{c}", name=f"xfm{c}")
                for c in range(nin)
            ]
            for c in range(nin):
                nc.sync.dma_start(xfm[c][:], P["xfm"][c])
            esrc_sb = load_const("esrc", dtype=I16)
            doff_sb = load_const("doff", dtype=BF16)
            stT_sb = load_const("stT", dtype=BF16)
            consts = {}
            for l in (1, 2, 3):
                for nm in (f"Wl{l}", f"Wr{l}"):
                    consts[nm] = load_const(nm, dtype=BF16, chunked=True)
                consts[f"attb{l}"] = load_const(f"attb{l}", dtype=BF16)
                if has_nbias[l - 1]:
                    for nm in (f"bl{l}", f"br{l}"):
                        consts[nm] = load_const(nm)
                if has_gbias[l - 1]:
                    consts[f"biasb{l}"] = load_const(f"biasb{l}")
            for nm in ("Wc1t", "Wc1b", "Wc2"):
                consts[nm] = load_const(nm, dtype=BF16, chunked=True)
            consts["Wc3"] = load_const("Wc3", dtype=BF16)
            if has_bc1:
                consts["bc1"] = load_const("bc1")
            for nm in ("bc2c", "nbc2c", "bc3c", "ones", "identf"):
                consts[nm] = load_const(nm)
            iotar = load_const("iotar", dtype=BF16)
            ones = consts["ones"]
            identf = consts["identf"]

            xr_w = cp.tile([128, wpc, HID], BF16, tag="xr_w")
            hbuf = cp.tile([128, wpc, HID], F32, tag="hbuf")
            b_win = cp.tile([128, wpc, HID], BF16, tag="b_win")

            # DRAM scratch
            xl_in = {
                l: dp.tile([L, HID], BF16, tag=f"xl_in{l}", name=f"xl_in{l}")
                for l in (1, 2, 3)
            }
            xl_full = {
                l: dp.tile([S, HID], BF16, tag=f"xl_full{l}",
                           name=f"xl_full{l}", addr_space="Shared")
                for l in (1, 2, 3)
            }
            a_in = dp.tile([L, HID], BF16, tag="a_in")
            a_full = dp.tile([S, HID], BF16, tag="a_full",
                             addr_space="Shared")

            def transposes(w):
                """hbuf[:, w, :] (f32) -> xfm chunks (bf16, transposed)."""
                ws = slice(w * 128, (w + 1) * 128)
                for c in range(2):
                    tp = psn.tile([128, 512], F32, tag="psm")
                    nc.tensor.transpose(
                        tp[:, :128], hbuf[:, w, c * 128 : (c + 1) * 128], identf[:]
                    )
                    nc.scalar.activation(xfm[c][:, ws], tp[:, :128], ACT.Copy)

            def node(l, w):
                """xl (-> DRAM) and xr (-> SBUF) node transforms for layer l."""
                ws = slice(w * 128, (w + 1) * 128)
                nk = nin if l == 1 else 2
                emit_bias = has_nbias[l - 1]
                for side, Wn, bn in (
                    ("l", f"Wl{l}", f"bl{l}"),
                    ("r", f"Wr{l}", f"br{l}"),
                ):
                    ps = psn.tile([128, 512], F32, tag="psm")
                    pz = ps[:, :HID]
                    for c in range(nk):
                        nc.tensor.matmul(
                            pz,
                            lhsT=xfm[c][:, ws],
                            rhs=consts[Wn][:, c, :],
                            start=(c == 0),
                            stop=(not emit_bias and c == nk - 1),
                        )
                    if emit_bias:
                        nc.tensor.matmul(
                            pz,
                            lhsT=ones[:1, 0:128],
                            rhs=consts[bn][:1, :],
                            start=False,
                            stop=True,
                        )
                    if side == "l":
                        xo = wp.tile([128, HID], BF16, tag="xo")
                        nc.scalar.activation(xo[:], pz, ACT.Copy)
                        nc.sync.dma_start(xl_in[l][ws, :], xo[:])
                    else:
                        nc.scalar.activation(xr_w[:, w, :], pz, ACT.Copy)

            def node_ab(w):
                """a = h3 @ Wc1[:256] (-> DRAM), b = h3 @ Wc1[256:] + bc1."""
                ws = slice(w * 128, (w + 1) * 128)
                ps = psn.tile([128, 512], F32, tag="psm")
                pa = ps[:, :HID]
                for c in range(2):
                    nc.tensor.matmul(
                        pa, lhsT=xfm[c][:, ws],
                        rhs=consts["Wc1t"][:, c, :],
                        start=(c == 0), stop=(c == 1),
                    )
                xo = wp.tile([128, HID], BF16, tag="xo")
                nc.scalar.activation(xo[:], pa, ACT.Copy)
                nc.sync.dma_start(a_in[ws, :], xo[:])
                ps2 = psn.tile([128, 512], F32, tag="psm")
                pb = ps2[:, :HID]
                for c in range(2):
                    nc.tensor.matmul(
                        pb, lhsT=xfm[c][:, ws],
                        rhs=consts["Wc1b"][:, c, :],
                        start=(c == 0), stop=(not has_bc1 and c == 1),
                    )
                if has_bc1:
                    nc.tensor.matmul(
                        pb, lhsT=ones[:1, 0:128],
                        rhs=consts["bc1"][:1, :],
                        start=False, stop=True,
                    )
                nc.scalar.activation(b_win[:, w, :], pb, ACT.Copy)

            def edge(l, w):
                """GATv2 edge phase for window w of layer l -> hbuf[:, w, :]."""
                ws = slice(w * 128, (w + 1) * 128)
                gt = ep.tile([128, T, HID], BF16, tag="gt")
                nc.gpsimd.dma_gather(
                    out_ap=gt[:, 0 : T - 1, :],
                    in_ap=xl_full[l][:],
                    idxs_ap=esrc_sb[:, w, :],
                    num_idxs=ewg,
                    num_idxs_reg=ewg,
                    elem_size=HID,
                    single_packet=False,
                )
                # self-loop tile: own window's xl, straight from local DRAM
                nc.sync.dma_start(gt[:, T - 1, :], xl_in[l][ws, :])
                # one-hot st[p, t, n] = (n == doff[p, t]) for the scatter
                st = ep.tile([128, T, 128], BF16, tag="st")
                nc.vector.tensor_tensor(
                    out=st[:],
                    in0=iotar[:].unsqueeze(1).to_broadcast([128, T, 128]),
                    in1=doff_sb[:, w, :].unsqueeze(2).to_broadcast([128, T, 128]),
                    op=ALU.is_equal,
                )
                # m = gt + xr[dst]; xr[dst] via stT one-hot matmuls (pairs)
                m = ep.tile([128, T, HID], BF16, tag="m")
                for tp in range(0, T, 2):
                    k = min(2, T - tp)
                    ps = psn.tile([128, 512], F32, tag="psm")
                    for i in range(k):
                        nc.tensor.matmul(
                            ps[:, i * HID : (i + 1) * HID],
                            lhsT=stT_sb[:, w, tp + i, :],
                            rhs=xr_w[:, w, :],
                            start=True,
                            stop=True,
                        )
                    nc.vector.tensor_tensor(
                        out=m[:, tp : tp + k, :],
                        in0=gt[:, tp : tp + k, :],
                        in1=ps[:, : k * HID].rearrange("p (k f) -> p k f", f=HID),
                        op=ALU.add,
                    )
                # leaky relu: max(x, 0.2x) on DVE (wx[:, :, :HID] as scratch)
                wx = ep.tile([128, T, HID + H], BF16, tag="wx")
                lk = wx[:, :, 0:HID]
                nc.vector.tensor_scalar(
                    out=lk, in0=m[:], scalar1=0.2, scalar2=None, op0=ALU.mult
                )
                nc.vector.tensor_tensor(out=m[:], in0=m[:], in1=lk, op=ALU.max)
                nc.vector.tensor_tensor(
                    out=m[:],
                    in0=m[:],
                    in1=consts[f"attb{l}"][:].unsqueeze(1).to_broadcast(
                        [128, T, HID]
                    ),
                    op=ALU.mult,
                )
                lg = ep.tile([128, T, H], F32, tag="lg")
                nc.vector.tensor_reduce(
                    out=lg[:].rearrange("p t h -> p (t h)"),
                    in_=m[:].rearrange("p t (g c) -> p (t g) c", c=C),
                    axis=mybir.AxisListType.X,
                    op=ALU.add,
                )
                nc.scalar.activation(wx[:, :, HID : HID + H], lg[:], ACT.Exp)
                nc.vector.tensor_tensor(
                    out=wx[:, :, 0:HID].rearrange("p t (h c) -> p t h c", c=C),
                    in0=gt[:].rearrange("p t (h c) -> p t h c", c=C),
                    in1=wx[:, :, HID : HID + H]
                    .unsqueeze(3)
                    .to_broadcast([128, T, H, C]),
                    op=ALU.mult,
                )
                ops = pso.tile([128, HID + H], F32, tag="ops")
                for t in range(T):
                    nc.tensor.matmul(
                        ops[:],
                        lhsT=st[:, t, :],
                        rhs=wx[:, t, :],
                        start=(t == 0),
                        stop=(t == T - 1),
                    )
                rc = ep.tile([128, H], F32, tag="rc")
                nc.vector.reciprocal(rc[:], ops[:, HID : HID + H])
                nc.vector.tensor_tensor(
                    out=hbuf[:, w, :].rearrange("p (h c) -> p h c", c=C),
                    in0=ops[:, 0:HID].rearrange("p (h c) -> p h c", c=C),
                    in1=rc[:].unsqueeze(2).to_broadcast([128, H, C]),
                    op=ALU.mult,
                )
                if has_gbias[l - 1]:
                    nc.vector.tensor_tensor(
                        out=hbuf[:, w, :],
                        in0=hbuf[:, w, :],
                        in1=consts[f"biasb{l}"][:],
                        op=ALU.add,
                    )
                if l <= 2:
                    # true ELU: exp(min(h,0)) - 1 + max(h,0) (no +1 offset --
                    # bf16 storage keeps full mantissa on the signal)
                    te = ep1.tile([128, HID], F32, tag="te")
                    nc.scalar.activation(
                        te[:], hbuf[:, w, :], ACT.Relu, scale=-1.0
                    )
                    nc.scalar.activation(te[:], te[:], ACT.Exp, scale=-1.0)
                    nc.vector.tensor_scalar(
                        out=te[:], in0=te[:], scalar1=-1.0, scalar2=None,
                        op0=ALU.add,
                    )
                    nc.scalar.activation(
                        hbuf[:, w, :], hbuf[:, w, :], ACT.Relu
                    )
                    nc.vector.tensor_add(hbuf[:, w, :], hbuf[:, w, :], te[:])

            def cls(w):
                """Edge classifier over window w's edge slots (feature-major)."""
                agT = ep.tile([128, HID // 128, ewg], BF16, tag="agT")
                nc.gpsimd.dma_gather(
                    out_ap=agT[:],
                    in_ap=a_full[:],
                    idxs_ap=esrc_sb[:, w, :],
                    num_idxs=ewg,
                    num_idxs_reg=ewg,
                    elem_size=HID,
                    transpose=True,
                    single_packet=False,
                )
                z1 = ep1.tile([128, 2, ewg], BF16, tag="z1")
                for mh in range(2):
                    for ch in range(NEC):
                        ps1 = psc.tile([128, EC], F32, tag="psc")
                        for i in range(EC // 128):
                            t = ch * (EC // 128) + i
                            nc.tensor.matmul(
                                ps1[:, i * 128 : (i + 1) * 128],
                                lhsT=b_win[:, w, mh * 128 : (mh + 1) * 128],
                                rhs=stT_sb[:, w, t, :],
                                start=True,
                                stop=True,
                            )
                        es = slice(ch * EC, (ch + 1) * EC)
                        sl = z1[:, mh, es]
                        nc.vector.tensor_tensor(
                            out=sl, in0=agT[:, mh, es], in1=ps1[:], op=ALU.add
                        )
                        # true ELU: (exp(min)-1) + max
                        t2 = ep1.tile([128, EC], F32, tag="t2")
                        nc.vector.tensor_scalar(
                            out=t2[:], in0=sl, scalar1=0.0, scalar2=None,
                            op0=ALU.min,
                        )
                        nc.scalar.activation(t2[:], t2[:], ACT.Exp)
                        nc.vector.tensor_scalar(
                            out=t2[:], in0=t2[:], scalar1=-1.0, scalar2=None,
                            op0=ALU.add,
                        )
                        nc.vector.tensor_scalar(
                            out=sl, in0=sl, scalar1=0.0, scalar2=None,
                            op0=ALU.max,
                        )
                        nc.vector.tensor_add(sl, sl, t2[:])
                z2 = ep1.tile([128, ewg], BF16, tag="z2")
                for ch in range(NEC):
                    es = slice(ch * EC, (ch + 1) * EC)
                    ps2 = psc.tile([128, EC], F32, tag="psc")
                    for mh in range(2):
                        nc.tensor.matmul(
                            ps2[:],
                            lhsT=consts["Wc2"][:, mh, :],
                            rhs=z1[:, mh, es],
                            start=(mh == 0),
                            stop=(mh == 1),
                        )
                    t2 = ep1.tile([128, EC], F32, tag="t2b")
                    nc.scalar.activation(
                        t2[:], ps2[:], ACT.Relu, scale=-1.0,
                        bias=consts["nbc2c"][:, 0:1],
                    )
                    nc.scalar.activation(t2[:], t2[:], ACT.Exp, scale=-1.0)
                    nc.vector.tensor_scalar(
                        out=t2[:], in0=t2[:], scalar1=-1.0, scalar2=None,
                        op0=ALU.add,
                    )
                    nc.scalar.activation(
                        z2[:, es], ps2[:], ACT.Relu, bias=consts["bc2c"][:, 0:1]
                    )
                    nc.vector.tensor_add(z2[:, es], z2[:, es], t2[:])
                zo = ep1.tile([NCLS, ewg], F32, tag="zo")
                for ch in range(NEC):
                    es = slice(ch * EC, (ch + 1) * EC)
                    ps3 = psc.tile([128, EC], F32, tag="psc")
                    nc.tensor.matmul(
                        ps3[:NCLS, :], lhsT=consts["Wc3"][:],
                        rhs=z2[:, es],
                        start=True, stop=True,
                    )
                    nc.scalar.activation(
                        zo[:, es], ps3[:NCLS, :], ACT.Identity,
                        bias=consts["bc3c"][:, 0:1],
                    )
                nc.sync.dma_start(out_t[:, w * ewg : (w + 1) * ewg], zo[:])

            # ================= schedule
            def ag(src_t, dst_t):
                nc.gpsimd.collective_compute(
                    "AllGather",
                    ALU.bypass,
                    replica_groups=rg,
                    ins=[src_t[:].opt()],
                    outs=[dst_t[:].opt()],
                )

            for w in range(wpc):
                node(1, w)
            ag(xl_in[1], xl_full[1])
            for l in (1, 2, 3):
                for w in range(wpc):
                    edge(l, w)
                    transposes(w)
                    if l < 3:
                        node(l + 1, w)
                    else:
                        node_ab(w)
                if l < 3:
                    ag(xl_in[l + 1], xl_full[l + 1])
                else:
                    ag(a_in, a_full)
            for w in range(wpc):
                cls(w)

    nc.compile()
    return nc


# ---------------------------------------------------------------- entry point

_CACHE = {}


def run(inputs, cfg, **kw):
    in_maps, meta = prepare_host(inputs, cfg)
    key = (cfg.n, cfg.e, cfg.wpc, cfg.T, cfg.in_dim, meta["flags"])
    if key not in _CACHE:
        _CACHE[key] = build_nc(cfg, flags=meta["flags"])
    nc = _CACHE[key]
    res = run_bass_kernel_spmd(nc, in_maps, list(range(NCORES)), **kw)
    out = np.zeros((cfg.e, NCLS), np.float32)
    for c in range(NCORES):
        o = np.asarray(res.results[c]["out_t"], np.float32)  # [5, wpc*ew]
        pos, orig = meta["out_maps"][c]
        out[orig] = o[:, pos].T
    return out, res


def kernel(**inputs) -> np.ndarray:
    n = inputs["x"].shape[0]
    e = inputs["edge_index"].shape[1]
    wpc = -(-n // (NCORES * 128))
    cfg = Cfg(n, e, wpc=wpc, T=9, in_dim=inputs["x"].shape[1])
    while True:
        try:
            out, _ = run(inputs, cfg)
            return out
        except AssertionError as ex:
            if "window overflow" in str(ex) and cfg.T < 16:
                cfg = Cfg(n, e, wpc=wpc, T=cfg.T + 1, in_dim=inputs["x"].shape[1])
                continue
            raise


# revision 23
# speedup vs baseline: 1.0653x; 1.0024x over previous
"""Trainium2 Bass kernel for nn_ScoreGraphReconstructor (3-layer GATv2 + edge MLP).

Sharding: nodes are permuted into 8*WPC windows of 128 slots, balanced by
in-degree. Each core owns WPC windows (contiguous slot range) and all edges
whose *target* lands in its windows, so the segment softmax/scatter is fully
core-local. Per layer: node matmuls on the local shard -> AllGather of the
source-side transform xl (bf16) -> edge phase (one dma_gather of xl[src],
xr[dst] reconstructed via one-hot transpose matmuls from SBUF, attention on
DVE/ACT, one-hot scatter matmul into PSUM). The edge classifier rides the same
window layout: per-node a/b = h3 @ Wc1 halves, AllGather a, transposed gather
of a[src] with the same indices, b[dst] via the same one-hots; z2/z3 are
feature-major matmuls. All matmuls bf16 (fp32 is 4x slower on the PE).
"""

import sys

for _p in ("/opt/trn_rl_repo",):
    if _p not in sys.path:
        sys.path.insert(0, _p)

import numpy as np
from ml_dtypes import bfloat16 as np_bf16

import concourse.bass as bass
import concourse.bacc as bacc
import concourse.mybir as mybir
import concourse.tile as tile
from concourse.bass_utils import run_bass_kernel_spmd

F32 = mybir.dt.float32
F32R = mybir.dt.float32r
BF16 = mybir.dt.bfloat16
I16 = mybir.dt.int16

NCORES = 8
H, C = 4, 64
HID = H * C  # 256
NCLS = 5
ACT = mybir.ActivationFunctionType
ALU = mybir.AluOpType


class Cfg:
    def __init__(self, n_nodes, n_edges, wpc, T, in_dim=256):
        self.n = n_nodes
        self.e = n_edges
        self.wpc = wpc                    # windows per core
        self.T = T                        # edge tiles (of 128) per window
        self.L = wpc * 128                # local slots per core
        self.S = NCORES * self.L          # total slots
        self.nwin = NCORES * self.wpc
        self.ew = T * 128                 # edge slots per window
        self.ewg = (T - 1) * 128          # gathered slots (last tile = self)
        self.in_dim = in_dim


# ---------------------------------------------------------------- host prep


def _balance_windows(deg, nwin):
    """Assign node n (with weight deg[n]) to one of nwin windows, each holding
    exactly 128 nodes (rest dummy), minimizing max window load. Greedy LPT."""
    import heapq

    n = len(deg)
    order = np.argsort(-deg, kind="stable")
    heap = [(0, w) for w in range(nwin)]
    heapq.heapify(heap)
    count = np.zeros(nwin, np.int64)
    slot_of = np.empty(n, np.int64)
    pos = np.zeros(nwin, np.int64)
    for node in order:
        while True:
            load, w = heapq.heappop(heap)
            if count[w] < 128:
                break
        slot_of[node] = w * 128 + pos[w]
        pos[w] += 1
        count[w] += 1
        if count[w] < 128:
            heapq.heappush(heap, (load + int(deg[node]), w))
    return slot_of


def _wrap_idx16(idx, rows=128):
    """dma_gather index layout: index i lives at [i % 16, i // 16] of a
    (rows, len/16) int16 SBUF tile; rows 16..127 padded with copies."""
    n = len(idx)
    assert n % 16 == 0
    blk = np.asarray(idx, np.int16).reshape(n // 16, 16).T
    return np.tile(blk, (rows // 16, 1))


def prepare_host(inputs, cfg):
    """Build per-core input maps + metadata. inputs: dict from setup_inputs."""
    cn = {k: np.asarray(v) for k, v in inputs.items()}
    x = cn["x"].astype(np.float32)
    ei = cn["edge_index"].astype(np.int64)
    row, col = ei[0], ei[1]
    n, e = cfg.n, cfg.e
    L, S, wpc, T = cfg.L, cfg.S, cfg.wpc, cfg.T
    ew = cfg.ew

    deg = np.bincount(col, minlength=n)
    slot_of = _balance_windows(deg, cfg.nwin)

    # --- edge lists (original edges only; self loops ride the synthesized
    # last tile of each window: doff = identity, xl via local DMA)
    src_sl = slot_of[row]
    dst_sl = slot_of[col]
    orig_id = np.arange(e, dtype=np.int64)
    win = dst_sl // 128
    ordr = np.argsort(win, kind="stable")
    src_sl, dst_sl, win, orig_id = (
        src_sl[ordr], dst_sl[ordr], win[ordr], orig_id[ordr]
    )
    starts = np.searchsorted(win, np.arange(cfg.nwin))
    ends = np.searchsorted(win, np.arange(cfg.nwin), side="right")
    maxcnt = int((ends - starts).max())
    assert maxcnt <= cfg.ewg, f"window overflow: {maxcnt} > {cfg.ewg}; raise T"

    # per-core edge tensors + output maps
    per_core = []
    out_maps = []
    ewg = cfg.ewg
    for c in range(NCORES):
        esrc = np.zeros((128, wpc, ewg // 16), np.int16)
        doff = np.full((128, wpc, T), 128, np.float32)
        poss, origs = [], []
        for wi in range(wpc):
            w = c * wpc + wi
            s0, s1 = starts[w], ends[w]
            cnt = s1 - s0
            srcw = np.zeros(ewg, np.int64)
            dofw = np.full(ewg, 128, np.int64)
            srcw[:cnt] = src_sl[s0:s1]
            dofw[:cnt] = dst_sl[s0:s1] - w * 128
            esrc[:, wi, :] = _wrap_idx16(srcw)
            # edge j -> partition j%128, tile j//128 (both gather + doff)
            doff[:, wi, : T - 1] = dofw.reshape(T - 1, 128).T
            doff[:, wi, T - 1] = np.arange(128)  # self-loop tile
            oid = orig_id[s0:s1]
            real = oid >= 0
            poss.append(wi * ewg + np.nonzero(real)[0])
            origs.append(oid[real])
        # one-hot transposed: stT[nslot, wi, t, p] = (doff[p, wi, t] == nslot)
        stT = (
            np.arange(128, dtype=np.int32)[:, None, None, None]
            == doff[None].transpose(0, 2, 3, 1)
        ).astype(np_bf16)
        per_core.append(
            {
                "esrc": esrc,
                "doff": doff.astype(np_bf16),
                "stT": stT,
            }
        )
        out_maps.append(
            (np.concatenate(poss), np.concatenate(origs))
        )

    # --- node features, permuted + transposed
    xp = np.zeros((S, cfg.in_dim), np.float32)
    xp[slot_of] = x
    nin = cfg.in_dim // 128

    # --- weights with ELU(-1) folds
    wmaps = {}
    for l in (1, 2, 3):
        kin = cfg.in_dim if l == 1 else HID
        Wl = cn[f"W{l}l"].astype(np.float32)
        Wr = cn[f"W{l}r"].astype(np.float32)
        bl = cn[f"b{l}l"].astype(np.float32).copy()
        br = cn[f"b{l}r"].astype(np.float32).copy()
        att = cn[f"att{l}"].astype(np.float32)
        wmaps[f"Wl{l}"] = Wl.reshape(kin // 128, 128, HID).astype(np_bf16)
        wmaps[f"Wr{l}"] = Wr.reshape(kin // 128, 128, HID).astype(np_bf16)
        wmaps[f"bl{l}"] = bl.reshape(1, HID)
        wmaps[f"br{l}"] = br.reshape(1, HID)
        wmaps[f"attb{l}"] = np.tile(att.reshape(1, HID), (128, 1)).astype(np_bf16)
        wmaps[f"biasb{l}"] = np.tile(
            cn[f"bias{l}"].astype(np.float32).reshape(1, HID), (128, 1)
        )
    Wc1 = cn["Wc1"].astype(np.float32)
    Wc2 = cn["Wc2"].astype(np.float32)
    Wc3 = cn["Wc3"].astype(np.float32)
    wmaps["Wc1t"] = Wc1[:HID].reshape(2, 128, HID).astype(np_bf16)
    wmaps["Wc1b"] = Wc1[HID:].reshape(2, 128, HID).astype(np_bf16)
    wmaps["bc1"] = cn["bc1"].astype(np.float32).reshape(1, HID)
    wmaps["Wc2"] = Wc2.reshape(2, 128, HID // 2).astype(np_bf16)
    wmaps["Wc3"] = Wc3.astype(np_bf16)  # [128, 5]
    bc2f = cn["bc2"].astype(np.float32)
    bc3f = cn["bc3"].astype(np.float32)
    wmaps["bc2c"] = bc2f.reshape(HID // 2, 1)
    wmaps["nbc2c"] = (-bc2f).reshape(HID // 2, 1)
    wmaps["bc3c"] = bc3f.reshape(NCLS, 1)
    wmaps["ones"] = np.ones((1, 512), np.float32)
    wmaps["identf"] = np.eye(128, dtype=np.float32)
    wmaps["iotar"] = np.tile(
        np.arange(128, dtype=np.float32), (128, 1)
    ).astype(np_bf16)

    in_maps = []
    for c in range(NCORES):
        m = dict(wmaps)
        m.update(per_core[c])
        xc = xp[c * L : (c + 1) * L].T.copy()  # (in_dim, L)
        m["xfm"] = xc.reshape(nin, 128, L).astype(np_bf16)
        in_maps.append(m)

    flags = (
        tuple(bool(np.any(cn[f"b{l}l"]) or np.any(cn[f"b{l}r"])) for l in (1, 2, 3)),
        tuple(bool(np.any(cn[f"bias{l}"])) for l in (1, 2, 3)),
        bool(np.any(cn["bc1"])),
    )
    meta = {"slot_of": slot_of, "cfg": cfg, "out_maps": out_maps,
            "flags": flags}
    return in_maps, meta


# ---------------------------------------------------------------- device build


def build_nc(cfg, flags=((True,) * 3, (True,) * 3, True), debug=False):
    nc = bacc.Bacc("TRN2", target_bir_lowering=False, debug=debug, num_devices=NCORES)
    has_nbias, has_gbias, has_bc1 = flags
    L, S, wpc, T, ew = cfg.L, cfg.S, cfg.wpc, cfg.T, cfg.ew
    ewg = cfg.ewg
    nin = cfg.in_dim // 128
    # classifier edge-chunk (z1/z2/z3 psum width)
    EC = 384 if ewg % 384 == 0 else (256 if ewg % 256 == 0 else 128)
    NEC = ewg // EC

    P = {}

    def pin(name, shape, dtype=F32):
        P[name] = nc.declare_dram_parameter(name, list(shape), dtype, isOutput=False)

    pin("xfm", (nin, 128, L), BF16)
    pin("esrc", (128, wpc, ewg // 16), I16)
    pin("doff", (128, wpc, T), BF16)
    pin("stT", (128, wpc, T, 128), BF16)
    for l in (1, 2, 3):
        nk = nin if l == 1 else 2
        pin(f"Wl{l}", (nk, 128, HID), BF16)
        pin(f"Wr{l}", (nk, 128, HID), BF16)
        pin(f"bl{l}", (1, HID))
        pin(f"br{l}", (1, HID))
        pin(f"attb{l}", (128, HID), BF16)
        pin(f"biasb{l}", (128, HID))
    pin("Wc1t", (2, 128, HID), BF16)
    pin("Wc1b", (2, 128, HID), BF16)
    pin("bc1", (1, HID))
    pin("Wc2", (2, 128, HID // 2), BF16)
    pin("Wc3", (128, NCLS), BF16)
    pin("bc2c", (HID // 2, 1))
    pin("nbc2c", (HID // 2, 1))
    pin("bc3c", (NCLS, 1))
    pin("ones", (1, 512))
    pin("identf", (128, 128))
    pin("iotar", (128, 128), BF16)
    out_t = nc.declare_dram_parameter(
        "out_t", [NCLS, wpc * ewg], F32, isOutput=True
    )

    rg = [list(range(NCORES))]

    with tile.TileContext(nc) as tc:
        with (
            tc.tile_pool(name="const", bufs=1) as cp,
            tc.tile_pool(name="dram", bufs=1, space="DRAM") as dp,
            tc.tile_pool(name="work", bufs=2) as wp,
            tc.tile_pool(name="epbig", bufs=4) as ep,
            tc.tile_pool(name="ep1", bufs=1) as ep1,
            tc.tile_pool(name="psn", bufs=2, space="PSUM") as psn,
            tc.tile_pool(name="pso", bufs=2, space="PSUM") as pso,
            tc.tile_pool(name="psc", bufs=2, space="PSUM") as psc,
        ):
            # ---------- constants into SBUF
            def load_const(name, dtype=F32, chunked=False):
                src = P[name]
                shp = list(src.shape)
                if chunked:
                    # [nk, 128, D] DRAM -> [128, nk, D] SBUF (weight chunks)
                    t = cp.tile([shp[1], shp[0], shp[2]], dtype, tag=name,
                                name=name + "_sb")
                    for c in range(shp[0]):
                        nc.sync.dma_start(t[:, c, :], src[c])
                else:
                    t = cp.tile(shp, dtype, tag=name, name=name + "_sb")
                    nc.sync.dma_start(t[:], src[:])
                return t

            xfm = [
                cp.tile([128, L], BF16, tag=f"xfm{c}", name=f"xfm{c}")
                for c in range(nin)
            ]
            for c in range(nin):
                nc.sync.dma_start(xfm[c][:], P["xfm"][c])
            esrc_sb = load_const("esrc", dtype=I16)
            doff_sb = load_const("doff", dtype=BF16)
            stT_sb = load_const("stT", dtype=BF16)
            consts = {}
            for l in (1, 2, 3):
                for nm in (f"Wl{l}", f"Wr{l}"):
                    consts[nm] = load_const(nm, dtype=BF16, chunked=True)
                consts[f"attb{l}"] = load_const(f"attb{l}", dtype=BF16)
                if has_nbias[l - 1]:
                    for nm in (f"bl{l}", f"br{l}"):
                        consts[nm] = load_const(nm)
                if has_gbias[l - 1]:
                    consts[f"biasb{l}"] = load_const(f"biasb{l}")
            for nm in ("Wc1t", "Wc1b", "Wc2"):
                consts[nm] = load_const(nm, dtype=BF16, chunked=True)
            consts["Wc3"] = load_const("Wc3", dtype=BF16)
            if has_bc1:
                consts["bc1"] = load_const("bc1")
            for nm in ("bc2c", "nbc2c", "bc3c", "ones", "identf"):
                consts[nm] = load_const(nm)
            iotar = load_const("iotar", dtype=BF16)
            ones = consts["ones"]
            identf = consts["identf"]

            xr_w = cp.tile([128, wpc, HID], BF16, tag="xr_w")
            hbuf = cp.tile([128, wpc, HID], F32, tag="hbuf")
            b_win = cp.tile([128, wpc, HID], BF16, tag="b_win")

            # DRAM scratch
            xl_in = {
                l: dp.tile([L, HID], BF16, tag=f"xl_in{l}", name=f"xl_in{l}")
                for l in (1, 2, 3)
            }
            xl_full = {
                l: dp.tile([S, HID], BF16, tag=f"xl_full{l}",
                           name=f"xl_full{l}", addr_space="Shared")
                for l in (1, 2, 3)
            }
            a_in = dp.tile([L, HID], BF16, tag="a_in")
            a_full = dp.tile([S, HID], BF16, tag="a_full",
                             addr_space="Shared")

            def transposes(w):
                """hbuf[:, w, :] (f32) -> xfm chunks (bf16, transposed)."""
                ws = slice(w * 128, (w + 1) * 128)
                for c in range(2):
                    tp = psn.tile([128, 512], F32, tag="psm")
                    nc.tensor.transpose(
                        tp[:, :128], hbuf[:, w, c * 128 : (c + 1) * 128], identf[:]
                    )
                    nc.scalar.activation(xfm[c][:, ws], tp[:, :128], ACT.Copy)

            def node(l, w):
                """xl (-> DRAM) and xr (-> SBUF) node transforms for layer l."""
                ws = slice(w * 128, (w + 1) * 128)
                nk = nin if l == 1 else 2
                emit_bias = has_nbias[l - 1]
                for side, Wn, bn in (
                    ("l", f"Wl{l}", f"bl{l}"),
                    ("r", f"Wr{l}", f"br{l}"),
                ):
                    ps = psn.tile([128, 512], F32, tag="psm")
                    pz = ps[:, :HID]
                    for c in range(nk):
                        nc.tensor.matmul(
                            pz,
                            lhsT=xfm[c][:, ws],
                            rhs=consts[Wn][:, c, :],
                            start=(c == 0),
                            stop=(not emit_bias and c == nk - 1),
                        )
                    if emit_bias:
                        nc.tensor.matmul(
                            pz,
                            lhsT=ones[:1, 0:128],
                            rhs=consts[bn][:1, :],
                            start=False,
                            stop=True,
                        )
                    if side == "l":
                        xo = wp.tile([128, HID], BF16, tag="xo")
                        nc.scalar.activation(xo[:], pz, ACT.Copy)
                        nc.sync.dma_start(xl_in[l][ws, :], xo[:])
                    else:
                        nc.scalar.activation(xr_w[:, w, :], pz, ACT.Copy)

            def node_ab(w):
                """a = h3 @ Wc1[:256] (-> DRAM), b = h3 @ Wc1[256:] + bc1."""
                ws = slice(w * 128, (w + 1) * 128)
                ps = psn.tile([128, 512], F32, tag="psm")
                pa = ps[:, :HID]
                for c in range(2):
                    nc.tensor.matmul(
                        pa, lhsT=xfm[c][:, ws],
                        rhs=consts["Wc1t"][:, c, :],
                        start=(c == 0), stop=(c == 1),
                    )
                xo = wp.tile([128, HID], BF16, tag="xo")
                nc.scalar.activation(xo[:], pa, ACT.Copy)
                nc.sync.dma_start(a_in[ws, :], xo[:])
                ps2 = psn.tile([128, 512], F32, tag="psm")
                pb = ps2[:, :HID]
                for c in range(2):
                    nc.tensor.matmul(
                        pb, lhsT=xfm[c][:, ws],
                        rhs=consts["Wc1b"][:, c, :],
                        start=(c == 0), stop=(not has_bc1 and c == 1),
                    )
                if has_bc1:
                    nc.tensor.matmul(
                        pb, lhsT=ones[:1, 0:128],
                        rhs=consts["bc1"][:1, :],
                        start=False, stop=True,
                    )
                nc.scalar.activation(b_win[:, w, :], pb, ACT.Copy)

            def edge(l, w):
                """GATv2 edge phase for window w of layer l -> hbuf[:, w, :]."""
                ws = slice(w * 128, (w + 1) * 128)
                gt = ep.tile([128, T, HID], BF16, tag="gt")
                nc.gpsimd.dma_gather(
                    out_ap=gt[:, 0 : T - 1, :],
                    in_ap=xl_full[l][:],
                    idxs_ap=esrc_sb[:, w, :],
                    num_idxs=ewg,
                    num_idxs_reg=ewg,
                    elem_size=HID,
                    single_packet=False,
                )
                # self-loop tile: own window's xl, straight from local DRAM
                nc.sync.dma_start(gt[:, T - 1, :], xl_in[l][ws, :])
                # one-hot st[p, t, n] = (n == doff[p, t]) for the scatter
                st = ep.tile([128, T, 128], BF16, tag="st")
                nc.vector.tensor_tensor(
                    out=st[:],
                    in0=iotar[:].unsqueeze(1).to_broadcast([128, T, 128]),
                    in1=doff_sb[:, w, :].unsqueeze(2).to_broadcast([128, T, 128]),
                    op=ALU.is_equal,
                )
                # m = gt + xr[dst]; xr[dst] via stT one-hot matmuls (pairs)
                m = ep.tile([128, T, HID], BF16, tag="m")
                for tp in range(0, T, 2):
                    k = min(2, T - tp)
                    ps = psn.tile([128, 512], F32, tag="psm")
                    for i in range(k):
                        nc.tensor.matmul(
                            ps[:, i * HID : (i + 1) * HID],
                            lhsT=stT_sb[:, w, tp + i, :],
                            rhs=xr_w[:, w, :],
                            start=True,
                            stop=True,
                        )
                    nc.vector.tensor_tensor(
                        out=m[:, tp : tp + k, :],
                        in0=gt[:, tp : tp + k, :],
                        in1=ps[:, : k * HID].rearrange("p (k f) -> p k f", f=HID),
                        op=ALU.add,
                    )
                # leaky relu: max(x, 0.2x) on DVE (wx[:, :, :HID] as scratch)
                wx = ep.tile([128, T, HID + H], BF16, tag="wx")
                lk = wx[:, :, 0:HID]
                nc.vector.tensor_scalar(
                    out=lk, in0=m[:], scalar1=0.2, scalar2=None, op0=ALU.mult
                )
                nc.vector.tensor_tensor(out=m[:], in0=m[:], in1=lk, op=ALU.max)
                nc.vector.tensor_tensor(
                    out=m[:],
                    in0=m[:],
                    in1=consts[f"attb{l}"][:].unsqueeze(1).to_broadcast(
                        [128, T, HID]
                    ),
                    op=ALU.mult,
                )
                lg = ep.tile([128, T, H], F32, tag="lg")
                nc.vector.tensor_reduce(
                    out=lg[:].rearrange("p t h -> p (t h)"),
                    in_=m[:].rearrange("p t (g c) -> p (t g) c", c=C),
                    axis=mybir.AxisListType.X,
                    op=ALU.add,
                )
                nc.scalar.activation(wx[:, :, HID : HID + H], lg[:], ACT.Exp)
                nc.vector.tensor_tensor(
                    out=wx[:, :, 0:HID].rearrange("p t (h c) -> p t h c", c=C),
                    in0=gt[:].rearrange("p t (h c) -> p t h c", c=C),
                    in1=wx[:, :, HID : HID + H]
                    .unsqueeze(3)
                    .to_broadcast([128, T, H, C]),
                    op=ALU.mult,
                )
                ops = pso.tile([128, HID + H], F32, tag="ops")
                for t in range(T):
                    nc.tensor.matmul(
                        ops[:],
                        lhsT=st[:, t, :],
                        rhs=wx[:, t, :],
                        start=(t == 0),
                        stop=(t == T - 1),
                    )
                rc = ep.tile([128, H], F32, tag="rc")
                nc.vector.reciprocal(rc[:], ops[:, HID : HID + H])
                nc.vector.tensor_tensor(
                    out=hbuf[:, w, :].rearrange("p (h c) -> p h c", c=C),
                    in0=ops[:, 0:HID].rearrange("p (h c) -> p h c", c=C),
                    in1=rc[:].unsqueeze(2).to_broadcast([128, H, C]),
                    op=ALU.mult,
                )
                if has_gbias[l - 1]:
                    nc.vector.tensor_tensor(
                        out=hbuf[:, w, :],
                        in0=hbuf[:, w, :],
                        in1=consts[f"biasb{l}"][:],
                        op=ALU.add,
                    )
                if l <= 2:
                    # true ELU: exp(min(h,0)) - 1 + max(h,0) (no +1 offset --
                    # bf16 storage keeps full mantissa on the signal)
                    te = ep1.tile([128, HID], F32, tag="te")
                    nc.scalar.activation(
                        te[:], hbuf[:, w, :], ACT.Relu, scale=-1.0
                    )
                    nc.scalar.activation(te[:], te[:], ACT.Exp, scale=-1.0)
                    nc.vector.tensor_scalar(
                        out=te[:], in0=te[:], scalar1=-1.0, scalar2=None,
                        op0=ALU.add,
                    )
                    nc.scalar.activation(
                        hbuf[:, w, :], hbuf[:, w, :], ACT.Relu
                    )
                    nc.vector.tensor_add(hbuf[:, w, :], hbuf[:, w, :], te[:])

            def cls(w):
                """Edge classifier over window w's edge slots (feature-major)."""
                agT = ep.tile([128, HID // 128, ewg], BF16, tag="agT")
                nc.gpsimd.dma_gather(
                    out_ap=agT[:],
                    in_ap=a_full[:],
                    idxs_ap=esrc_sb[:, w, :],
                    num_idxs=ewg,
                    num_idxs_reg=ewg,
                    elem_size=HID,
                    transpose=True,
                    single_packet=False,
                )
                z1 = ep1.tile([128, 2, ewg], BF16, tag="z1")
                for mh in range(2):
                    for ch in range(NEC):
                        ps1 = psc.tile([128, EC], F32, tag="psc")
                        for i in range(EC // 128):
                            t = ch * (EC // 128) + i
                            nc.tensor.matmul(
                                ps1[:, i * 128 : (i + 1) * 128],
                                lhsT=b_win[:, w, mh * 128 : (mh + 1) * 128],
                                rhs=stT_sb[:, w, t, :],
                                start=True,
                                stop=True,
                            )
                        es = slice(ch * EC, (ch + 1) * EC)
                        sl = z1[:, mh, es]
                        nc.vector.tensor_tensor(
                            out=sl, in0=agT[:, mh, es], in1=ps1[:], op=ALU.add
                        )
                        # true ELU: (exp(min)-1) + max
                        t2 = ep1.tile([128, EC], F32, tag="t2")
                        nc.vector.tensor_scalar(
                            out=t2[:], in0=sl, scalar1=0.0, scalar2=None,
                            op0=ALU.min,
                        )
                        nc.scalar.activation(t2[:], t2[:], ACT.Exp)
                        nc.vector.tensor_scalar(
                            out=t2[:], in0=t2[:], scalar1=-1.0, scalar2=None,
                            op0=ALU.add,
                        )
                        nc.vector.tensor_scalar(
                            out=sl, in0=sl, scalar1=0.0, scalar2=None,
                            op0=ALU.max,
                        )
                        nc.vector.tensor_add(sl, sl, t2[:])
                z2 = ep1.tile([128, ewg], BF16, tag="z2")
                for ch in range(NEC):
                    es = slice(ch * EC, (ch + 1) * EC)
                    ps2 = psc.tile([128, EC], F32, tag="psc")
                    for mh in range(2):
                        nc.tensor.matmul(
                            ps2[:],
                            lhsT=consts["Wc2"][:, mh, :],
                            rhs=z1[:, mh, es],
                            start=(mh == 0),
                            stop=(mh == 1),
                        )
                    t2 = ep1.tile([128, EC], F32, tag="t2b")
                    nc.scalar.activation(
                        t2[:], ps2[:], ACT.Relu, scale=-1.0,
                        bias=consts["nbc2c"][:, 0:1],
                    )
                    nc.scalar.activation(t2[:], t2[:], ACT.Exp, scale=-1.0)
                    nc.vector.tensor_scalar(
                        out=t2[:], in0=t2[:], scalar1=-1.0, scalar2=None,
                        op0=ALU.add,
                    )
                    nc.scalar.activation(
                        z2[:, es], ps2[:], ACT.Relu, bias=consts["bc2c"][:, 0:1]
                    )
                    nc.vector.tensor_add(z2[:, es], z2[:, es], t2[:])
                zo = ep1.tile([NCLS, ewg], F32, tag="zo")
                for ch in range(NEC):
                    es = slice(ch * EC, (ch + 1) * EC)
                    ps3 = psc.tile([128, EC], F32, tag="psc")
                    nc.tensor.matmul(
                        ps3[:NCLS, :], lhsT=consts["Wc3"][:],
                        rhs=z2[:, es],
                        start=True, stop=True,
                    )
                    nc.scalar.activation(
                        zo[:, es], ps3[:NCLS, :], ACT.Identity,
                        bias=consts["bc3c"][:, 0:1],
                    )
                nc.sync.dma_start(out_t[:, w * ewg : (w + 1) * ewg], zo[:])

            # ================= schedule
            def ag(src_t, dst_t):
                nc.gpsimd.collective_compute(
                    "AllGather",
                    ALU.bypass,
                    replica_groups=rg,
                    ins=[src_t[:].opt()],
                    outs=[dst_t[:].opt()],
                )

            for w in range(wpc):
                node(1, w)
            ag(xl_in[1], xl_full[1])
            for l in (1, 2, 3):
                for w in range(wpc):
                    edge(l, w)
                    transposes(w)
                    if l < 3:
                        node(l + 1, w)
                    else:
                        node_ab(w)
                if l < 3:
                    ag(xl_in[l + 1], xl_full[l + 1])
                else:
                    ag(a_in, a_full)
            for w in range(wpc):
                cls(w)

    nc.compile()
    return nc


# ---------------------------------------------------------------- entry point

_CACHE = {}


def run(inputs, cfg, **kw):
    in_maps, meta = prepare_host(inputs, cfg)
    key = (cfg.n, cfg.e, cfg.wpc, cfg.T, cfg.in_dim, meta["flags"])
    if key not in _CACHE:
        _CACHE[key] = build_nc(cfg, flags=meta["flags"])
    nc = _CACHE[key]
    res = run_bass_kernel_spmd(nc, in_maps, list(range(NCORES)), **kw)
    out = np.zeros((cfg.e, NCLS), np.float32)
    for c in range(NCORES):
        o = np.asarray(res.results[c]["out_t"], np.float32)  # [5, wpc*ew]
        pos, orig = meta["out_maps"][c]
        out[orig] = o[:, pos].T
    return out, res


def kernel(**inputs) -> np.ndarray:
    n = inputs["x"].shape[0]
    e = inputs["edge_index"].shape[1]
    wpc = -(-n // (NCORES * 128))
    cfg = Cfg(n, e, wpc=wpc, T=9, in_dim=inputs["x"].shape[1])
    while True:
        try:
            out, _ = run(inputs, cfg)
            return out
        except AssertionError as ex:
            if "window overflow" in str(ex) and cfg.T < 16:
                cfg = Cfg(n, e, wpc=wpc, T=cfg.T + 1, in_dim=inputs["x"].shape[1])
                continue
            raise
